# revision 11
# baseline (speedup 1.0000x reference)
"""Trainium2 Bass kernel for nn_Attention_84567906058480.

Multi-head attention (B=4, T=2048, C=1024, H=16, D=64) on 8 NeuronCores.

Sharding: core c = (batch b = c//2, head-group hg = c%2).  Each core computes
Q/K/V for its 8 heads over its batch (tensor-parallel split of wq/wk/wv rows),
runs attention, applies its column-slice of wo to get a partial output, and a
pairwise ReduceScatter (groups [2b, 2b+1]) sums the two head-group partials
while scattering token halves: the even core ends with tokens [0,1024) of its
batch, the odd core with tokens [1024,2048).  The host concatenates.

Implementation notes:
- Activations/weights run bf16 on the PE (f32 PSUM accumulate); rel-err ~5e-3.
- Weights are pre-transposed (and wq pre-scaled by 1/sqrt(D)) on the host and
  shipped bf16, so only x needs on-chip transposes.
- Scores are computed directly transposed (S.T = k.T-tiles @ qT) so no
  P-transpose is needed; the softmax denominator comes from a ones-column
  appended to V (M=65 stationary operand); exp needs no max-subtraction
  (|scores| < ~3 by construction).
- Every attention matmul contracts over K=128 (per-head K tensors are
  zero-padded into the other head's partition range) so the PE array never
  switches tiling modes; score and output matmul emission is software-
  pipelined (outputs lag scores by 2 iterations, normalization is deferred
  past the next head's start) to keep the PE stream stall-free.
- The output bias bo is halved on the host so the pairwise reduce adds it
  exactly once.
"""

import os
import sys
import types
import contextlib

import numpy as np

if "/opt/trn_rl_repo" not in sys.path:
    sys.path.insert(0, "/opt/trn_rl_repo")

import ml_dtypes
import concourse.bass as bass  # noqa: F401
import concourse.mybir as mybir
import concourse.tile as tile
from concourse import bacc
from concourse import bass_utils
from concourse.masks import make_identity

F32 = mybir.dt.float32
BF16 = mybir.dt.bfloat16
AF = mybir.ActivationFunctionType

B, T, C = 4, 2048, 1024
H, D = 16, 64
HPC = 8            # heads per core
FS = HPC * D       # per-core feature shard = 512
N_CORES = 8
PAIRS = [[0, 1], [2, 3], [4, 5], [6, 7]]

NT = T // 128      # 16 token tiles
NCT = C // 128     # 8 contraction tiles
NFB = FS // 128    # 4 feature blocks per core
QW = 512           # q chunk width
NQC = T // QW      # 4 q chunks


def _emit(nc, tc, x_ext, wqt_ext, wkt_ext, wvt_ext, wot_ext, bo_ext, out_ext):
    with tc.tile_pool(name="const", bufs=1) as constp, \
         tc.tile_pool(name="persist", bufs=1) as pp:

        # ---- constants -------------------------------------------------
        identb = constp.tile([128, 128], BF16, tag="identb")
        make_identity(nc, identb[:, :])
        ones_col = constp.tile([1, 128], F32, tag="ones")
        nc.vector.memset(ones_col[:, :], 1.0)
        Emat = constp.tile([128, 64], BF16, tag="Emat")
        nc.vector.memset(Emat[:, :], 0.0)
        nc.vector.memset(Emat[0:1, :], 1.0)
        bo_row = constp.tile([1, C], F32, tag="bo_row")
        nc.sync.dma_start(bo_row[:, :], bo_ext[:].unsqueeze(0))
        bo_bcast = constp.tile([128, C], F32, tag="bo_bcast")

        # ---- persistent activation storage (bf16) ----------------------
        qT = [pp.tile([128, T], BF16, tag=f"qT{fb}", name=f"qT{fb}") for fb in range(NFB)]
        kTh = [pp.tile([128, T], BF16, tag=f"kTh{h}", name=f"kTh{h}") for h in range(HPC)]
        v_ext = [pp.tile([128, HPC * 65], BF16, tag=f"vx{tt}", name=f"vx{tt}") for tt in range(NT)]
        woT = [pp.tile([128, C], BF16, tag=f"woT{fb}", name=f"woT{fb}") for fb in range(NFB)]
        lout = [pp.tile([128, T], BF16, tag=f"lo{fb}", name=f"lo{fb}") for fb in range(NFB)]

        # =================================================================
        # Phase B/C: weight loads, x transposes, QKV projections
        # =================================================================
        with tc.tile_pool(name="pbc", bufs=2) as pbc, \
             tc.tile_pool(name="ps_tr", bufs=4, space="PSUM") as ps_tr, \
             tc.tile_pool(name="ps_acc", bufs=2, space="PSUM") as ps_acc:

            # bias broadcast [128, C] via rank-1 ones matmul (exact f32)
            for cc in range(2):
                bb = ps_acc.tile([128, 512], F32, tag="acc")
                nc.tensor.matmul(bb[:, :], ones_col[:, :],
                                 bo_row[:, cc * 512:(cc + 1) * 512],
                                 start=True, stop=True)
                nc.vector.tensor_copy(bo_bcast[:, cc * 512:(cc + 1) * 512], bb[:, :])

            # ---- weights: direct (host-pre-transposed) loads ------------
            def ctile_major(ext):
                return ext[:].rearrange("(ct p) f -> p ct f", p=128)

            wqTf = pbc.tile([128, NCT * FS], BF16, tag="wqTf", bufs=1)
            nc.sync.dma_start(wqTf[:].rearrange("p (ct f) -> p ct f", f=FS),
                              ctile_major(wqt_ext))
            wkTf = pbc.tile([128, NCT * FS], BF16, tag="wkTf", bufs=1)
            nc.sync.dma_start(wkTf[:].rearrange("p (ct f) -> p ct f", f=FS),
                              ctile_major(wkt_ext))
            wvT = pbc.tile([128, NCT * FS], BF16, tag="wvT", bufs=1)
            nc.sync.dma_start(wvT[:].rearrange("p (ct f) -> p ct f", f=FS),
                              ctile_major(wvt_ext))
            for fb in range(NFB):
                nc.sync.dma_start(woT[fb][:, :], wot_ext[fb * 128:(fb + 1) * 128, :])

            # ---- xT: transpose x (bf16) into [C-part, tok] --------------
            xT = [pbc.tile([128, T], BF16, tag=f"xT{ct}", name=f"xT{ct}", bufs=1) for ct in range(NCT)]
            for tt in range(NT):
                xnat = pbc.tile([128, C], BF16, tag="xnat", bufs=3)
                nc.sync.dma_start(xnat[:, :], x_ext[tt * 128:(tt + 1) * 128, :])
                for ct in range(NCT):
                    tr = ps_tr.tile([128, 128], BF16, tag="tr")
                    nc.tensor.transpose(tr[:, :], xnat[:, ct * 128:(ct + 1) * 128],
                                        identb[:, :])
                    nc.vector.tensor_copy(xT[ct][:, tt * 128:(tt + 1) * 128], tr[:, :])

            # ---- q/k projections ---------------------------------------
            # kTh[h]: head h's k at partitions (h%2)*64..+64, zeros in the
            # other half -> K=128 score matmuls with the full-qT rhs.
            for h in range(HPC):
                z0 = (1 - (h % 2)) * 64
                nc.vector.memset(kTh[h][z0:z0 + 64, :], 0.0)
            for fb in range(NFB):
                for name, wf in (("wq", wqTf), ("wk", wkTf)):
                    for tch in range(NQC):
                        acc = ps_acc.tile([128, QW], F32, tag="acc")
                        for ct in range(NCT):
                            nc.tensor.matmul(
                                acc[:, :],
                                wf[:, ct * FS + fb * 128: ct * FS + fb * 128 + 128],
                                xT[ct][:, tch * QW:(tch + 1) * QW],
                                start=(ct == 0), stop=(ct == NCT - 1))
                        if name == "wq":
                            nc.vector.tensor_copy(
                                qT[fb][:, tch * QW:(tch + 1) * QW], acc[:, :])
                        else:
                            for hh in range(2):
                                nc.vector.tensor_copy(
                                    kTh[fb * 2 + hh][hh * 64:(hh + 1) * 64,
                                                     tch * QW:(tch + 1) * QW],
                                    acc[hh * 64:(hh + 1) * 64, :])

            # ---- v: natural [tok, feat] with ones column interleave -----
            for tt in range(NT):
                acc = ps_acc.tile([128, FS], F32, tag="acc")
                for ct in range(NCT):
                    nc.tensor.matmul(
                        acc[:, :],
                        xT[ct][:, tt * 128:(tt + 1) * 128],
                        wvT[:, ct * FS:(ct + 1) * FS],
                        start=(ct == 0), stop=(ct == NCT - 1))
                nc.vector.memset(v_ext[tt][:, :], 1.0)
                dst = v_ext[tt][:].rearrange("p (h e) -> p h e", e=65)[:, :, 0:64]
                src = acc[:].rearrange("p (h e) -> p h e", e=64)
                nc.vector.tensor_copy(dst, src)

        # =================================================================
        # Phase D/E: attention + output projection + ReduceScatter
        # =================================================================
        with tc.tile_pool(name="pd", bufs=4) as pd, \
             tc.tile_pool(name="pdram", bufs=4, space="DRAM") as pdram, \
             tc.tile_pool(name="ps_sT", bufs=1, space="PSUM") as ps_sT, \
             tc.tile_pool(name="ps_oT", bufs=2, space="PSUM") as ps_oT, \
             tc.tile_pool(name="ps_rb", bufs=1, space="PSUM") as ps_rb, \
             tc.tile_pool(name="ps_pj", bufs=1, space="PSUM") as ps_pj:

            r_pad = pd.tile([128, QW], BF16, tag="r_pad", bufs=1, name="r_pad")
            nc.vector.memset(r_pad[:, :], 0.0)
            # one sT tile for the whole phase: continuous ping-pong across
            # heads (bank-level WAR tracking), no per-head drain barrier
            sT = ps_sT.tile([128, 2048], F32, tag="sT", name="sT")
            gkp = [0]

            LAG = 2  # outT matmuls run LAG kp-iterations behind sT/exp
            pending_norm = []

            def attn(h, qc):
                fb, hh = divmod(h, 2)
                q_ap = qT[fb][:, qc * QW:(qc + 1) * QW]
                outT = ps_oT.tile([65, QW], F32, tag="outT")
                NKP = NT // 2
                pTs = {}

                def emit_outT(kp):
                    for j in range(2):
                        kt = kp * 2 + j
                        nc.tensor.matmul(
                            outT[:, :],
                            v_ext[kt][:, h * 65:(h + 1) * 65],
                            pTs[kp][:, j * 512:(j + 1) * 512],
                            start=(kp == 0 and j == 0),
                            stop=(kp == NKP - 1 and j == 1))

                for kp in range(NKP):
                    off = (gkp[0] % 2) * 1024
                    gkp[0] += 1
                    for j in range(2):
                        kt = kp * 2 + j
                        nc.tensor.matmul(
                            sT[:, off + j * 512: off + j * 512 + 512],
                            kTh[h][:, kt * 128:(kt + 1) * 128],
                            q_ap, start=True, stop=True)
                    pT = pd.tile([128, 1024], BF16, tag="pT")
                    nc.scalar.activation(pT[:, :], sT[:, off:off + 1024], AF.Exp)
                    pTs[kp] = pT
                    if kp == 1 and pending_norm:
                        # previous head's normalization: inputs long since
                        # ready; emitting here keeps the PE stream stall-free
                        pending_norm.pop(0)()
                    if kp >= LAG:
                        emit_outT(kp - LAG)
                        del pTs[kp - LAG]
                for kp in range(NKP - LAG, NKP):
                    emit_outT(kp)

                def norm():
                    # denominators -> reciprocal -> broadcast -> normalize
                    r_row = pd.tile([1, QW], F32, tag="r_row")
                    nc.vector.reciprocal(r_row[:, :], outT[64:65, :])
                    nc.vector.tensor_copy(r_pad[0:1, :], r_row[:, :])
                    rb_ps = ps_rb.tile([64, QW], F32, tag="rb")
                    nc.tensor.matmul(rb_ps[:, :], Emat[:, :], r_pad[:, :],
                                     start=True, stop=True)
                    rb = pd.tile([64, QW], F32, tag="rb_sb")
                    nc.vector.tensor_copy(rb[:, :], rb_ps[:, :])
                    nc.vector.tensor_mul(
                        lout[fb][hh * 64:(hh + 1) * 64, qc * QW:(qc + 1) * QW],
                        outT[0:64, :], rb[:, :])
                pending_norm.append(norm)

            rs_mode = os.environ.get("KERNEL_RS_MODE", "one")
            if rs_mode == "one":
                rs_in_big = pdram.tile([T, C], BF16, tag="rs_in_big")
                rs_out_big = pdram.tile([T // 2, C], BF16, tag="rs_out_big")
            rs_chunks_done = []

            def proj_chunk(tq, rs_in, row_base):
                for half in range(2):
                    for t2 in range(2):
                        tok0 = half * 1024 + tq * 256 + t2 * 128
                        for cc in range(2):
                            pj = ps_pj.tile([128, 512], F32, tag="pj")
                            for fb in range(NFB):
                                nc.tensor.matmul(
                                    pj[:, :],
                                    lout[fb][:, tok0:tok0 + 128],
                                    woT[fb][:, cc * 512:(cc + 1) * 512],
                                    start=(fb == 0), stop=(fb == NFB - 1))
                            ot = pd.tile([128, 512], BF16, tag="ot")
                            nc.vector.tensor_add(
                                ot[:, :], pj[:, :],
                                bo_bcast[:, cc * 512:(cc + 1) * 512])
                            r0 = row_base(half) + t2 * 128
                            nc.sync.dma_start(
                                rs_in[r0:r0 + 128, cc * 512:(cc + 1) * 512],
                                ot[:, :])

            def drain_rows(rs_out, src_row0, out_row0, nrows):
                for t2 in range(nrows // 128):
                    fo_bf = pd.tile([128, C], BF16, tag="fo_bf")
                    nc.sync.dma_start(
                        fo_bf[:, :],
                        rs_out[src_row0 + t2 * 128: src_row0 + (t2 + 1) * 128, :])
                    fo = pd.tile([128, C], F32, tag="fo")
                    nc.vector.tensor_copy(fo[:, :], fo_bf[:, :])
                    nc.sync.dma_start(
                        out_ext[out_row0 + t2 * 128: out_row0 + (t2 + 1) * 128, :],
                        fo[:, :])

            def proj_rs(tq):
                phases = os.environ.get("KERNEL_PHASES", "full")
                if rs_mode == "one":
                    proj_chunk(tq, rs_in_big,
                               lambda half: half * 1024 + tq * 256)
                    rs_chunks_done.append(tq)
                    if len(rs_chunks_done) == 4:
                        if phases == "nors":
                            nc.sync.dma_start(rs_out_big[:, :], rs_in_big[0:T // 2, :])
                        else:
                            nc.gpsimd.collective_compute(
                                "ReduceScatter", mybir.AluOpType.add,
                                replica_groups=PAIRS,
                                ins=[rs_in_big.opt()], outs=[rs_out_big.opt()])
                        drain_rows(rs_out_big, 0, 0, T // 2)
                else:
                    rs_in = pdram.tile([512, C], BF16, tag="rs_in", name="rs_in")
                    rs_out = pdram.tile([256, C], BF16, tag="rs_out", name="rs_out")
                    proj_chunk(tq, rs_in, lambda half: half * 256)
                    if phases == "nors":
                        nc.sync.dma_start(rs_out[:, :], rs_in[0:256, :])
                    else:
                        nc.gpsimd.collective_compute(
                            "ReduceScatter", mybir.AluOpType.add,
                            replica_groups=PAIRS,
                            ins=[rs_in.opt()], outs=[rs_out.opt()])
                    drain_rows(rs_out, 0, tq * 256, 256)

            phases = os.environ.get("KERNEL_PHASES", "full")
            if phases == "qkv":
                dbg = pd.tile([128, C], F32, tag="dbg")
                nc.vector.tensor_copy(dbg[:, :], qT[0][:, 0:1024])
                nc.sync.dma_start(out_ext[0:128, :], dbg[:, :])
            else:
                for pair_i, (qca, qcb) in enumerate(((0, 2), (1, 3))):
                    for qc in (qca, qcb):
                        for fb in range(NFB):
                            for hh in range(2):
                                attn(fb * 2 + hh, qc)
                    while pending_norm:
                        pending_norm.pop(0)()
                    if phases != "attn":
                        for tq in (pair_i * 2, pair_i * 2 + 1):
                            proj_rs(tq)
                if phases == "attn":
                    dbg = pd.tile([128, C], F32, tag="dbg")
                    for fb in range(NFB):
                        nc.vector.tensor_copy(dbg[:, :], lout[fb][:, 0:1024])
                        nc.sync.dma_start(out_ext[fb * 128:(fb + 1) * 128, :], dbg[:, :])


def _build_nc():
    nc = bacc.Bacc("TRN2", target_bir_lowering=False, debug=False,
                   num_devices=N_CORES)
    x_ext = nc.dram_tensor("x", [T, C], BF16, kind="ExternalInput")
    wqt_ext = nc.dram_tensor("wqt", [C, FS], BF16, kind="ExternalInput")
    wkt_ext = nc.dram_tensor("wkt", [C, FS], BF16, kind="ExternalInput")
    wvt_ext = nc.dram_tensor("wvt", [C, FS], BF16, kind="ExternalInput")
    wot_ext = nc.dram_tensor("wot", [FS, C], BF16, kind="ExternalInput")
    bo_ext = nc.dram_tensor("bo", [C], F32, kind="ExternalInput")
    out_ext = nc.dram_tensor("out", [T // 2, C], F32, kind="ExternalOutput")
    with tile.TileContext(nc) as tc:
        _emit(nc, tc, x_ext, wqt_ext, wkt_ext, wvt_ext, wot_ext, bo_ext, out_ext)
    nc.finalize()
    return nc


# ---------------------------------------------------------------------------
# NTFF profiling under axon (used when KERNEL_TRACE=1): the agent image's
# antenv lacks axon_hooks, so inject an equivalent module backed by the
# libaxon_pjrt.so profiling C ABI.
# ---------------------------------------------------------------------------
def _ensure_axon_hooks():
    try:
        from antenv.axon_hooks import get_axon_ntff_profile_hook  # noqa: F401
        return
    except ImportError:
        pass
    import ctypes
    import antenv

    so_path = "/opt/axon/libaxon_pjrt.so"
    lib = ctypes.CDLL(so_path)
    if not hasattr(lib, "axon_start_nrt_profile"):
        return
    lib.axon_start_nrt_profile.argtypes = [ctypes.POINTER(ctypes.c_int64),
                                           ctypes.c_size_t]
    lib.axon_start_nrt_profile.restype = ctypes.c_int64
    lib.axon_stop_nrt_profile.argtypes = [ctypes.c_char_p]
    lib.axon_stop_nrt_profile.restype = ctypes.c_int64

    @contextlib.contextmanager
    def _hook(output_dir, device_ids):
        import jax
        jax.devices()
        if device_ids:
            ids = (ctypes.c_int64 * len(device_ids))(*device_ids)
            rc = lib.axon_start_nrt_profile(ids, len(device_ids))
        else:
            rc = lib.axon_start_nrt_profile(None, 0)
        if rc != 0:
            raise RuntimeError(f"axon_start_nrt_profile rc={rc}")
        try:
            yield
        finally:
            n = lib.axon_stop_nrt_profile(str(output_dir).encode())
            print(f"ntff profile: {n} file(s) -> {output_dir}", file=sys.stderr)

    holder = [_hook]
    mod = types.ModuleType("antenv.axon_hooks")
    mod.get_axon_ntff_profile_hook = lambda: holder[0]
    mod.set_axon_ntff_profile_hook = lambda h: holder.__setitem__(0, h)
    sys.modules["antenv.axon_hooks"] = mod
    antenv.axon_hooks = mod
    # avoid S3 upload attempts during profile post-processing
    bass_utils.upload_artifacts = lambda tmpdir: f"(local:{tmpdir})"


_NC = None
LAST = {}


def kernel(hidden_states, wq, wk, wv, wo, bo):
    global _NC
    hidden_states = np.asarray(hidden_states, dtype=np.float32)
    wq = np.asarray(wq, dtype=np.float32)
    wk = np.asarray(wk, dtype=np.float32)
    wv = np.asarray(wv, dtype=np.float32)
    wo = np.asarray(wo, dtype=np.float32)
    bo = np.asarray(bo, dtype=np.float32)

    if _NC is None:
        _NC = _build_nc()

    bf = ml_dtypes.bfloat16
    scale = np.float32(D ** -0.5)
    in_maps = []
    for c in range(N_CORES):
        b, hg = divmod(c, 2)
        fr = hg * FS
        in_maps.append({
            "x": np.ascontiguousarray(hidden_states[b]).astype(bf),
            "wqt": np.ascontiguousarray((wq[fr:fr + FS] * scale).T).astype(bf),
            "wkt": np.ascontiguousarray(wk[fr:fr + FS].T).astype(bf),
            "wvt": np.ascontiguousarray(wv[fr:fr + FS].T).astype(bf),
            "wot": np.ascontiguousarray(wo[:, fr:fr + FS].T).astype(bf),
            "bo": bo * np.float32(0.5),
        })

    trace = os.environ.get("KERNEL_TRACE", "0") == "1"
    if trace:
        _ensure_axon_hooks()
    res = bass_utils.run_bass_kernel_spmd(
        _NC, in_maps, core_ids=list(range(N_CORES)), trace=trace)
    LAST["exec_time_ns"] = res.exec_time_ns
    LAST["res"] = res

    y = np.empty((B, T, C), dtype=np.float32)
    for c in range(N_CORES):
        b, hg = divmod(c, 2)
        y[b, hg * (T // 2):(hg + 1) * (T // 2), :] = res.results[c]["out"]
    return y


# revision 12
# speedup vs baseline: 1.0741x; 1.0741x over previous
"""Trainium2 Bass kernel for nn_Attention_84567906058480.

Multi-head attention (B=4, T=2048, C=1024, H=16, D=64) on 8 NeuronCores.

Sharding: core c = (batch b = c//2, head-group hg = c%2).  Each core computes
Q/K/V for its 8 heads over its batch (tensor-parallel split of wq/wk/wv rows),
runs attention, applies its column-slice of wo to get a partial output, and a
pairwise ReduceScatter (groups [2b, 2b+1]) sums the two head-group partials
while scattering token halves: the even core ends with tokens [0,1024) of its
batch, the odd core with tokens [1024,2048).  The host concatenates.

Implementation notes:
- Activations/weights run bf16 on the PE (f32 PSUM accumulate); rel-err ~5e-3.
- Weights are pre-transposed (and wq pre-scaled by 1/sqrt(D)) on the host and
  shipped bf16, so only x needs on-chip transposes.
- Scores are computed directly transposed (S.T = k.T-tiles @ qT) so no
  P-transpose is needed; the softmax denominator comes from a ones-column
  appended to V (M=65 stationary operand); exp needs no max-subtraction
  (|scores| < ~3 by construction).
- Every attention matmul contracts over K=128 (per-head K tensors are
  zero-padded into the other head's partition range) so the PE array never
  switches tiling modes; score and output matmul emission is software-
  pipelined (outputs lag scores by 2 iterations, normalization is deferred
  past the next head's start) to keep the PE stream stall-free.
- The output bias bo is halved on the host so the pairwise reduce adds it
  exactly once.
"""

import os
import sys
import types
import contextlib

import numpy as np

if "/opt/trn_rl_repo" not in sys.path:
    sys.path.insert(0, "/opt/trn_rl_repo")

import ml_dtypes
import concourse.bass as bass  # noqa: F401
import concourse.mybir as mybir
import concourse.tile as tile
from concourse import bacc
from concourse import bass_utils
from concourse.masks import make_identity

F32 = mybir.dt.float32
BF16 = mybir.dt.bfloat16
AF = mybir.ActivationFunctionType

B, T, C = 4, 2048, 1024
H, D = 16, 64
HPC = 8            # heads per core
FS = HPC * D       # per-core feature shard = 512
N_CORES = 8
PAIRS = [[0, 1], [2, 3], [4, 5], [6, 7]]

NT = T // 128      # 16 token tiles
NCT = C // 128     # 8 contraction tiles
NFB = FS // 128    # 4 feature blocks per core
QW = 512           # q chunk width
NQC = T // QW      # 4 q chunks


def _emit(nc, tc, x_ext, wqt_ext, wkt_ext, wvt_ext, wot_ext, bo_ext, out_ext):
    with tc.tile_pool(name="const", bufs=1) as constp, \
         tc.tile_pool(name="persist", bufs=1) as pp:

        # ---- constants -------------------------------------------------
        identb = constp.tile([128, 128], BF16, tag="identb")
        make_identity(nc, identb[:, :])
        ones_col = constp.tile([1, 128], F32, tag="ones")
        nc.vector.memset(ones_col[:, :], 1.0)
        Emat = constp.tile([128, 64], BF16, tag="Emat")
        nc.vector.memset(Emat[:, :], 0.0)
        nc.vector.memset(Emat[0:1, :], 1.0)
        bo_row = constp.tile([1, C], F32, tag="bo_row")
        nc.sync.dma_start(bo_row[:, :], bo_ext[:].unsqueeze(0))
        bo_bcast = constp.tile([128, C], F32, tag="bo_bcast")

        # ---- persistent activation storage (bf16) ----------------------
        qT = [pp.tile([128, T], BF16, tag=f"qT{fb}", name=f"qT{fb}") for fb in range(NFB)]
        kTh = [pp.tile([128, T], BF16, tag=f"kTh{h}", name=f"kTh{h}") for h in range(HPC)]
        v_ext = [pp.tile([128, HPC * 65], BF16, tag=f"vx{tt}", name=f"vx{tt}") for tt in range(NT)]
        woT = [pp.tile([128, C], BF16, tag=f"woT{fb}", name=f"woT{fb}") for fb in range(NFB)]
        lout = [pp.tile([128, T], BF16, tag=f"lo{fb}", name=f"lo{fb}") for fb in range(NFB)]

        # =================================================================
        # Phase B/C: weight loads, x transposes, QKV projections
        # =================================================================
        with tc.tile_pool(name="pbc", bufs=2) as pbc, \
             tc.tile_pool(name="ps_tr", bufs=4, space="PSUM") as ps_tr, \
             tc.tile_pool(name="ps_acc", bufs=2, space="PSUM") as ps_acc:

            # bias broadcast [128, C] via rank-1 ones matmul (exact f32)
            for cc in range(2):
                bb = ps_acc.tile([128, 512], F32, tag="acc")
                nc.tensor.matmul(bb[:, :], ones_col[:, :],
                                 bo_row[:, cc * 512:(cc + 1) * 512],
                                 start=True, stop=True)
                nc.vector.tensor_copy(bo_bcast[:, cc * 512:(cc + 1) * 512], bb[:, :])

            # ---- weights: direct (host-pre-transposed) loads ------------
            def ctile_major(ext):
                return ext[:].rearrange("(ct p) f -> p ct f", p=128)

            wqTf = pbc.tile([128, NCT * FS], BF16, tag="wqTf", bufs=1)
            nc.sync.dma_start(wqTf[:].rearrange("p (ct f) -> p ct f", f=FS),
                              ctile_major(wqt_ext))
            wkTf = pbc.tile([128, NCT * FS], BF16, tag="wkTf", bufs=1)
            nc.sync.dma_start(wkTf[:].rearrange("p (ct f) -> p ct f", f=FS),
                              ctile_major(wkt_ext))
            wvT = pbc.tile([128, NCT * FS], BF16, tag="wvT", bufs=1)
            nc.sync.dma_start(wvT[:].rearrange("p (ct f) -> p ct f", f=FS),
                              ctile_major(wvt_ext))
            for fb in range(NFB):
                nc.sync.dma_start(woT[fb][:, :], wot_ext[fb * 128:(fb + 1) * 128, :])

            # ---- xT: transpose x (bf16) into [C-part, tok] --------------
            xT = [pbc.tile([128, T], BF16, tag=f"xT{ct}", name=f"xT{ct}", bufs=1) for ct in range(NCT)]
            for tt in range(NT):
                xnat = pbc.tile([128, C], BF16, tag="xnat", bufs=3)
                nc.sync.dma_start(xnat[:, :], x_ext[tt * 128:(tt + 1) * 128, :])
                for ct in range(NCT):
                    tr = ps_tr.tile([128, 128], BF16, tag="tr")
                    nc.tensor.transpose(tr[:, :], xnat[:, ct * 128:(ct + 1) * 128],
                                        identb[:, :])
                    nc.vector.tensor_copy(xT[ct][:, tt * 128:(tt + 1) * 128], tr[:, :])

            # ---- q/k projections ---------------------------------------
            # kTh[h]: head h's k at partitions (h%2)*64..+64, zeros in the
            # other half -> K=128 score matmuls with the full-qT rhs.
            for h in range(HPC):
                z0 = (1 - (h % 2)) * 64
                nc.vector.memset(kTh[h][z0:z0 + 64, :], 0.0)
            for fb in range(NFB):
                for name, wf in (("wq", wqTf), ("wk", wkTf)):
                    for tch in range(NQC):
                        acc = ps_acc.tile([128, QW], F32, tag="acc")
                        for ct in range(NCT):
                            nc.tensor.matmul(
                                acc[:, :],
                                wf[:, ct * FS + fb * 128: ct * FS + fb * 128 + 128],
                                xT[ct][:, tch * QW:(tch + 1) * QW],
                                start=(ct == 0), stop=(ct == NCT - 1))
                        if name == "wq":
                            nc.vector.tensor_copy(
                                qT[fb][:, tch * QW:(tch + 1) * QW], acc[:, :])
                        else:
                            for hh in range(2):
                                nc.vector.tensor_copy(
                                    kTh[fb * 2 + hh][hh * 64:(hh + 1) * 64,
                                                     tch * QW:(tch + 1) * QW],
                                    acc[hh * 64:(hh + 1) * 64, :])

            # ---- v: natural [tok, feat] with ones column interleave -----
            for tt in range(NT):
                acc = ps_acc.tile([128, FS], F32, tag="acc")
                for ct in range(NCT):
                    nc.tensor.matmul(
                        acc[:, :],
                        xT[ct][:, tt * 128:(tt + 1) * 128],
                        wvT[:, ct * FS:(ct + 1) * FS],
                        start=(ct == 0), stop=(ct == NCT - 1))
                nc.vector.memset(v_ext[tt][:, :], 1.0)
                dst = v_ext[tt][:].rearrange("p (h e) -> p h e", e=65)[:, :, 0:64]
                src = acc[:].rearrange("p (h e) -> p h e", e=64)
                nc.vector.tensor_copy(dst, src)

        # =================================================================
        # Phase D/E: attention + output projection + ReduceScatter
        # =================================================================
        with tc.tile_pool(name="pd", bufs=4) as pd, \
             tc.tile_pool(name="pdram", bufs=4, space="DRAM") as pdram, \
             tc.tile_pool(name="ps_sT", bufs=1, space="PSUM") as ps_sT, \
             tc.tile_pool(name="ps_oT", bufs=2, space="PSUM") as ps_oT, \
             tc.tile_pool(name="ps_rb", bufs=1, space="PSUM") as ps_rb, \
             tc.tile_pool(name="ps_pj", bufs=1, space="PSUM") as ps_pj:

            l_pad = pd.tile([128, QW], BF16, tag="l_pad", bufs=1, name="l_pad")
            nc.vector.memset(l_pad[:, :], 0.0)
            # one sT tile for the whole phase: continuous ping-pong across
            # heads (bank-level WAR tracking), no per-head drain barrier
            sT = ps_sT.tile([128, 2048], F32, tag="sT", name="sT")
            gkp = [0]

            LAG = 3  # outT matmuls run LAG kp-iterations behind sT/exp
            pending_norm = []

            def attn(h, qc):
                fb, hh = divmod(h, 2)
                q_ap = qT[fb][:, qc * QW:(qc + 1) * QW]
                outT = ps_oT.tile([65, QW], F32, tag="outT")
                NKP = NT // 2
                pTs = {}

                def emit_outT(kp):
                    for j in range(2):
                        kt = kp * 2 + j
                        nc.tensor.matmul(
                            outT[:, :],
                            v_ext[kt][:, h * 65:(h + 1) * 65],
                            pTs[kp][:, j * 512:(j + 1) * 512],
                            start=(kp == 0 and j == 0),
                            stop=(kp == NKP - 1 and j == 1))

                for kp in range(NKP):
                    off = (gkp[0] % 2) * 1024
                    gkp[0] += 1
                    for j in range(2):
                        kt = kp * 2 + j
                        nc.tensor.matmul(
                            sT[:, off + j * 512: off + j * 512 + 512],
                            kTh[h][:, kt * 128:(kt + 1) * 128],
                            q_ap, start=True, stop=True)
                    pT = pd.tile([128, 1024], BF16, tag="pT", bufs=5)
                    nc.scalar.activation(pT[:, :], sT[:, off:off + 1024], AF.Exp)
                    pTs[kp] = pT
                    if kp == 1 and pending_norm:
                        # previous head's normalization: inputs long since
                        # ready; emitting here keeps the PE stream stall-free
                        pending_norm.pop(0)()
                    if kp >= LAG:
                        emit_outT(kp - LAG)
                        del pTs[kp - LAG]
                for kp in range(NKP - LAG, NKP):
                    emit_outT(kp)

                def norm():
                    # broadcast denominators l across 64 partitions via the
                    # one-hot-row matmul, then a partition-parallel reciprocal
                    # (a [1,512] DVE op runs on one lane = ~3.4us; avoid it)
                    nc.scalar.copy(l_pad[0:1, :], outT[64:65, :])
                    rb_ps = ps_rb.tile([64, QW], F32, tag="rb")
                    nc.tensor.matmul(rb_ps[:, :], Emat[:, :], l_pad[:, :],
                                     start=True, stop=True)
                    rb = pd.tile([64, QW], F32, tag="rb_sb")
                    nc.vector.reciprocal(rb[:, :], rb_ps[:, :])
                    nc.vector.tensor_mul(
                        lout[fb][hh * 64:(hh + 1) * 64, qc * QW:(qc + 1) * QW],
                        outT[0:64, :], rb[:, :])
                pending_norm.append(norm)

            rs_mode = os.environ.get("KERNEL_RS_MODE", "one")
            if rs_mode == "one":
                rs_in_big = pdram.tile([T, C], BF16, tag="rs_in_big")
                rs_out_big = pdram.tile([T // 2, C], BF16, tag="rs_out_big")
            rs_chunks_done = []

            def proj_chunk(tq, rs_in, row_base):
                for half in range(2):
                    for t2 in range(2):
                        tok0 = half * 1024 + tq * 256 + t2 * 128
                        for cc in range(2):
                            pj = ps_pj.tile([128, 512], F32, tag="pj")
                            for fb in range(NFB):
                                nc.tensor.matmul(
                                    pj[:, :],
                                    lout[fb][:, tok0:tok0 + 128],
                                    woT[fb][:, cc * 512:(cc + 1) * 512],
                                    start=(fb == 0), stop=(fb == NFB - 1))
                            ot = pd.tile([128, 512], BF16, tag="ot")
                            nc.vector.tensor_add(
                                ot[:, :], pj[:, :],
                                bo_bcast[:, cc * 512:(cc + 1) * 512])
                            r0 = row_base(half) + t2 * 128
                            nc.sync.dma_start(
                                rs_in[r0:r0 + 128, cc * 512:(cc + 1) * 512],
                                ot[:, :])

            def drain_rows(rs_out, src_row0, out_row0, nrows):
                for t2 in range(nrows // 128):
                    fo_bf = pd.tile([128, C], BF16, tag="fo_bf")
                    nc.sync.dma_start(
                        fo_bf[:, :],
                        rs_out[src_row0 + t2 * 128: src_row0 + (t2 + 1) * 128, :])
                    fo = pd.tile([128, C], F32, tag="fo")
                    nc.vector.tensor_copy(fo[:, :], fo_bf[:, :])
                    nc.sync.dma_start(
                        out_ext[out_row0 + t2 * 128: out_row0 + (t2 + 1) * 128, :],
                        fo[:, :])

            def proj_rs(tq):
                phases = os.environ.get("KERNEL_PHASES", "full")
                if rs_mode == "one":
                    proj_chunk(tq, rs_in_big,
                               lambda half: half * 1024 + tq * 256)
                    rs_chunks_done.append(tq)
                    if len(rs_chunks_done) == 4:
                        if phases == "nors":
                            nc.sync.dma_start(rs_out_big[:, :], rs_in_big[0:T // 2, :])
                        else:
                            nc.gpsimd.collective_compute(
                                "ReduceScatter", mybir.AluOpType.add,
                                replica_groups=PAIRS,
                                ins=[rs_in_big.opt()], outs=[rs_out_big.opt()])
                        drain_rows(rs_out_big, 0, 0, T // 2)
                else:
                    rs_in = pdram.tile([512, C], BF16, tag="rs_in", name="rs_in")
                    rs_out = pdram.tile([256, C], BF16, tag="rs_out", name="rs_out")
                    proj_chunk(tq, rs_in, lambda half: half * 256)
                    if phases == "nors":
                        nc.sync.dma_start(rs_out[:, :], rs_in[0:256, :])
                    else:
                        nc.gpsimd.collective_compute(
                            "ReduceScatter", mybir.AluOpType.add,
                            replica_groups=PAIRS,
                            ins=[rs_in.opt()], outs=[rs_out.opt()])
                    drain_rows(rs_out, 0, tq * 256, 256)

            phases = os.environ.get("KERNEL_PHASES", "full")
            if phases == "qkv":
                dbg = pd.tile([128, C], F32, tag="dbg")
                nc.vector.tensor_copy(dbg[:, :], qT[0][:, 0:1024])
                nc.sync.dma_start(out_ext[0:128, :], dbg[:, :])
            else:
                for pair_i, (qca, qcb) in enumerate(((0, 2), (1, 3))):
                    for qc in (qca, qcb):
                        for fb in range(NFB):
                            for hh in range(2):
                                attn(fb * 2 + hh, qc)
                    while pending_norm:
                        pending_norm.pop(0)()
                    if phases != "attn":
                        for tq in (pair_i * 2, pair_i * 2 + 1):
                            proj_rs(tq)
                if phases == "attn":
                    dbg = pd.tile([128, C], F32, tag="dbg")
                    for fb in range(NFB):
                        nc.vector.tensor_copy(dbg[:, :], lout[fb][:, 0:1024])
                        nc.sync.dma_start(out_ext[fb * 128:(fb + 1) * 128, :], dbg[:, :])


def _build_nc():
    nc = bacc.Bacc("TRN2", target_bir_lowering=False, debug=False,
                   num_devices=N_CORES)
    x_ext = nc.dram_tensor("x", [T, C], BF16, kind="ExternalInput")
    wqt_ext = nc.dram_tensor("wqt", [C, FS], BF16, kind="ExternalInput")
    wkt_ext = nc.dram_tensor("wkt", [C, FS], BF16, kind="ExternalInput")
    wvt_ext = nc.dram_tensor("wvt", [C, FS], BF16, kind="ExternalInput")
    wot_ext = nc.dram_tensor("wot", [FS, C], BF16, kind="ExternalInput")
    bo_ext = nc.dram_tensor("bo", [C], F32, kind="ExternalInput")
    out_ext = nc.dram_tensor("out", [T // 2, C], F32, kind="ExternalOutput")
    with tile.TileContext(nc) as tc:
        _emit(nc, tc, x_ext, wqt_ext, wkt_ext, wvt_ext, wot_ext, bo_ext, out_ext)
    nc.finalize()
    return nc


# ---------------------------------------------------------------------------
# NTFF profiling under axon (used when KERNEL_TRACE=1): the agent image's
# antenv lacks axon_hooks, so inject an equivalent module backed by the
# libaxon_pjrt.so profiling C ABI.
# ---------------------------------------------------------------------------
def _ensure_axon_hooks():
    try:
        from antenv.axon_hooks import get_axon_ntff_profile_hook  # noqa: F401
        return
    except ImportError:
        pass
    import ctypes
    import antenv

    so_path = "/opt/axon/libaxon_pjrt.so"
    lib = ctypes.CDLL(so_path)
    if not hasattr(lib, "axon_start_nrt_profile"):
        return
    lib.axon_start_nrt_profile.argtypes = [ctypes.POINTER(ctypes.c_int64),
                                           ctypes.c_size_t]
    lib.axon_start_nrt_profile.restype = ctypes.c_int64
    lib.axon_stop_nrt_profile.argtypes = [ctypes.c_char_p]
    lib.axon_stop_nrt_profile.restype = ctypes.c_int64

    @contextlib.contextmanager
    def _hook(output_dir, device_ids):
        import jax
        jax.devices()
        if device_ids:
            ids = (ctypes.c_int64 * len(device_ids))(*device_ids)
            rc = lib.axon_start_nrt_profile(ids, len(device_ids))
        else:
            rc = lib.axon_start_nrt_profile(None, 0)
        if rc != 0:
            raise RuntimeError(f"axon_start_nrt_profile rc={rc}")
        try:
            yield
        finally:
            n = lib.axon_stop_nrt_profile(str(output_dir).encode())
            print(f"ntff profile: {n} file(s) -> {output_dir}", file=sys.stderr)

    holder = [_hook]
    mod = types.ModuleType("antenv.axon_hooks")
    mod.get_axon_ntff_profile_hook = lambda: holder[0]
    mod.set_axon_ntff_profile_hook = lambda h: holder.__setitem__(0, h)
    sys.modules["antenv.axon_hooks"] = mod
    antenv.axon_hooks = mod
    # avoid S3 upload attempts during profile post-processing
    bass_utils.upload_artifacts = lambda tmpdir: f"(local:{tmpdir})"


_NC = None
LAST = {}


def kernel(hidden_states, wq, wk, wv, wo, bo):
    global _NC
    hidden_states = np.asarray(hidden_states, dtype=np.float32)
    wq = np.asarray(wq, dtype=np.float32)
    wk = np.asarray(wk, dtype=np.float32)
    wv = np.asarray(wv, dtype=np.float32)
    wo = np.asarray(wo, dtype=np.float32)
    bo = np.asarray(bo, dtype=np.float32)

    if _NC is None:
        _NC = _build_nc()

    bf = ml_dtypes.bfloat16
    scale = np.float32(D ** -0.5)
    in_maps = []
    for c in range(N_CORES):
        b, hg = divmod(c, 2)
        fr = hg * FS
        in_maps.append({
            "x": np.ascontiguousarray(hidden_states[b]).astype(bf),
            "wqt": np.ascontiguousarray((wq[fr:fr + FS] * scale).T).astype(bf),
            "wkt": np.ascontiguousarray(wk[fr:fr + FS].T).astype(bf),
            "wvt": np.ascontiguousarray(wv[fr:fr + FS].T).astype(bf),
            "wot": np.ascontiguousarray(wo[:, fr:fr + FS].T).astype(bf),
            "bo": bo * np.float32(0.5),
        })

    trace = os.environ.get("KERNEL_TRACE", "0") == "1"
    if trace:
        _ensure_axon_hooks()
    res = bass_utils.run_bass_kernel_spmd(
        _NC, in_maps, core_ids=list(range(N_CORES)), trace=trace)
    LAST["exec_time_ns"] = res.exec_time_ns
    LAST["res"] = res

    y = np.empty((B, T, C), dtype=np.float32)
    for c in range(N_CORES):
        b, hg = divmod(c, 2)
        y[b, hg * (T // 2):(hg + 1) * (T // 2), :] = res.results[c]["out"]
    return y


# revision 13
# speedup vs baseline: 1.3776x; 1.2825x over previous
"""Trainium2 Bass kernel for nn_Attention_84567906058480.

Multi-head attention (B=4, T=2048, C=1024, H=16, D=64) on 8 NeuronCores.

Sharding: core c = (batch b = c//2, head-group hg = c%2).  Each core computes
Q/K/V for its 8 heads over its batch (tensor-parallel split of wq/wk/wv rows),
runs attention, applies its column-slice of wo to get a partial output, and a
pairwise ReduceScatter (groups [2b, 2b+1]) sums the two head-group partials
while scattering token halves: the even core ends with tokens [0,1024) of its
batch, the odd core with tokens [1024,2048).  The host concatenates.

Implementation notes:
- Activations/weights run bf16 on the PE (f32 PSUM accumulate); rel-err ~5e-3.
- Weights are pre-transposed (and wq pre-scaled by 1/sqrt(D)) on the host and
  shipped bf16, so only x needs on-chip transposes.
- Scores are computed directly transposed (S.T = k.T-tiles @ qT) so no
  P-transpose is needed; the softmax denominator comes from a ones-column
  appended to V (M=65 stationary operand); exp needs no max-subtraction
  (|scores| < ~3 by construction).
- Every attention matmul contracts over K=128 (per-head K tensors are
  zero-padded into the other head's partition range) so the PE array never
  switches tiling modes; score and output matmul emission is software-
  pipelined (outputs lag scores by 2 iterations, normalization is deferred
  past the next head's start) to keep the PE stream stall-free.
- The output bias bo is halved on the host so the pairwise reduce adds it
  exactly once.
"""

import os
import sys
import types
import contextlib

import numpy as np

if "/opt/trn_rl_repo" not in sys.path:
    sys.path.insert(0, "/opt/trn_rl_repo")

import ml_dtypes
import concourse.bass as bass  # noqa: F401
import concourse.mybir as mybir
import concourse.tile as tile
from concourse import bacc
from concourse import bass_utils
from concourse.masks import make_identity

F32 = mybir.dt.float32
BF16 = mybir.dt.bfloat16
AF = mybir.ActivationFunctionType

B, T, C = 4, 2048, 1024
H, D = 16, 64
HPC = 8            # heads per core
FS = HPC * D       # per-core feature shard = 512
N_CORES = 8
PAIRS = [[0, 1], [2, 3], [4, 5], [6, 7]]

NT = T // 128      # 16 token tiles
NCT = C // 128     # 8 contraction tiles
NFB = FS // 128    # 4 feature blocks per core
QW = 512           # q chunk width
NQC = T // QW      # 4 q chunks


def _emit(nc, tc, x_ext, wqt_ext, wkt_ext, wvt_ext, wot_ext, bo_ext, out_ext):
    with tc.tile_pool(name="const", bufs=1) as constp, \
         tc.tile_pool(name="persist", bufs=1) as pp:

        # ---- constants -------------------------------------------------
        identb = constp.tile([128, 128], BF16, tag="identb")
        make_identity(nc, identb[:, :])
        ones_col = constp.tile([1, 128], F32, tag="ones")
        nc.vector.memset(ones_col[:, :], 1.0)
        Emat = constp.tile([128, 64], BF16, tag="Emat")
        nc.vector.memset(Emat[:, :], 0.0)
        nc.vector.memset(Emat[0:1, :], 1.0)
        bo_row = constp.tile([1, C], F32, tag="bo_row")
        nc.sync.dma_start(bo_row[:, :], bo_ext[:].unsqueeze(0))
        bo_bcast = constp.tile([128, C], F32, tag="bo_bcast")

        # ---- persistent activation storage (bf16) ----------------------
        qT = [pp.tile([128, T], BF16, tag=f"qT{fb}", name=f"qT{fb}") for fb in range(NFB)]
        kTh = [pp.tile([128, T], BF16, tag=f"kTh{h}", name=f"kTh{h}") for h in range(HPC)]
        v_ext = [pp.tile([128, HPC * 65], BF16, tag=f"vx{tt}", name=f"vx{tt}") for tt in range(NT)]
        woT = [pp.tile([128, C], BF16, tag=f"woT{fb}", name=f"woT{fb}") for fb in range(NFB)]
        lout = [pp.tile([128, T], BF16, tag=f"lo{fb}", name=f"lo{fb}") for fb in range(NFB)]

        # =================================================================
        # Phase B/C: weight loads, x transposes, QKV projections
        # =================================================================
        with tc.tile_pool(name="pbc", bufs=2) as pbc, \
             tc.tile_pool(name="ps_tr", bufs=4, space="PSUM") as ps_tr, \
             tc.tile_pool(name="ps_acc", bufs=2, space="PSUM") as ps_acc:

            # bias broadcast [128, C] via rank-1 ones matmul (exact f32)
            for cc in range(2):
                bb = ps_acc.tile([128, 512], F32, tag="acc")
                nc.tensor.matmul(bb[:, :], ones_col[:, :],
                                 bo_row[:, cc * 512:(cc + 1) * 512],
                                 start=True, stop=True)
                nc.vector.tensor_copy(bo_bcast[:, cc * 512:(cc + 1) * 512], bb[:, :])

            # ---- weights: direct (host-pre-transposed) loads ------------
            def ctile_major(ext):
                return ext[:].rearrange("(ct p) f -> p ct f", p=128)

            # ---- xT: transpose x (bf16) into [C-part, tok] --------------
            # weight DMAs are interleaved after the first x tiles so the PE
            # gets transpose work immediately
            xT = [pbc.tile([128, T], BF16, tag=f"xT{ct}", name=f"xT{ct}", bufs=1) for ct in range(NCT)]
            wqTf = pbc.tile([128, NCT * FS], BF16, tag="wqTf", bufs=1)
            wkTf = pbc.tile([128, NCT * FS], BF16, tag="wkTf", bufs=1)
            wvT = pbc.tile([128, NCT * FS], BF16, tag="wvT", bufs=1)
            for tt in range(NT):
                xnat = pbc.tile([128, C], BF16, tag="xnat", bufs=3)
                nc.sync.dma_start(xnat[:, :], x_ext[tt * 128:(tt + 1) * 128, :])
                if tt == 2:
                    nc.sync.dma_start(wvT[:].rearrange("p (ct f) -> p ct f", f=FS),
                                      ctile_major(wvt_ext))
                elif tt == 4:
                    nc.sync.dma_start(wqTf[:].rearrange("p (ct f) -> p ct f", f=FS),
                                      ctile_major(wqt_ext))
                elif tt == 6:
                    nc.sync.dma_start(wkTf[:].rearrange("p (ct f) -> p ct f", f=FS),
                                      ctile_major(wkt_ext))
                elif tt == 8:
                    for fb in range(NFB):
                        nc.sync.dma_start(woT[fb][:, :],
                                          wot_ext[fb * 128:(fb + 1) * 128, :])
                for ct in range(NCT):
                    tr = ps_tr.tile([128, 128], BF16, tag="tr")
                    nc.tensor.transpose(tr[:, :], xnat[:, ct * 128:(ct + 1) * 128],
                                        identb[:, :])
                    nc.vector.tensor_copy(xT[ct][:, tt * 128:(tt + 1) * 128], tr[:, :])

            # ---- q/k projections ---------------------------------------
            # kTh[h]: head h's k at partitions (h%2)*64..+64, zeros in the
            # other half -> K=128 score matmuls with the full-qT rhs.
            for h in range(HPC):
                z0 = (1 - (h % 2)) * 64
                nc.vector.memset(kTh[h][z0:z0 + 64, :], 0.0)
            for fb in range(NFB):
                for name, wf in (("wq", wqTf), ("wk", wkTf)):
                    for tch in range(NQC):
                        acc = ps_acc.tile([128, QW], F32, tag="acc")
                        for ct in range(NCT):
                            nc.tensor.matmul(
                                acc[:, :],
                                wf[:, ct * FS + fb * 128: ct * FS + fb * 128 + 128],
                                xT[ct][:, tch * QW:(tch + 1) * QW],
                                start=(ct == 0), stop=(ct == NCT - 1))
                        if name == "wq":
                            nc.vector.tensor_copy(
                                qT[fb][:, tch * QW:(tch + 1) * QW], acc[:, :])
                        else:
                            for hh in range(2):
                                nc.vector.tensor_copy(
                                    kTh[fb * 2 + hh][hh * 64:(hh + 1) * 64,
                                                     tch * QW:(tch + 1) * QW],
                                    acc[hh * 64:(hh + 1) * 64, :])

            # ---- v: natural [tok, feat] with ones column interleave -----
            for tt in range(NT):
                acc = ps_acc.tile([128, FS], F32, tag="acc")
                for ct in range(NCT):
                    nc.tensor.matmul(
                        acc[:, :],
                        xT[ct][:, tt * 128:(tt + 1) * 128],
                        wvT[:, ct * FS:(ct + 1) * FS],
                        start=(ct == 0), stop=(ct == NCT - 1))
                nc.vector.memset(v_ext[tt][:, :], 1.0)
                dst = v_ext[tt][:].rearrange("p (h e) -> p h e", e=65)[:, :, 0:64]
                src = acc[:].rearrange("p (h e) -> p h e", e=64)
                nc.vector.tensor_copy(dst, src)

        # =================================================================
        # Phase D/E: attention + output projection + ReduceScatter
        # =================================================================
        with tc.tile_pool(name="pd", bufs=4) as pd, \
             tc.tile_pool(name="pdram", bufs=4, space="DRAM") as pdram, \
             tc.tile_pool(name="ps_sT", bufs=1, space="PSUM") as ps_sT, \
             tc.tile_pool(name="ps_oT", bufs=2, space="PSUM") as ps_oT, \
             tc.tile_pool(name="ps_rb", bufs=1, space="PSUM") as ps_rb, \
             tc.tile_pool(name="ps_pj", bufs=1, space="PSUM") as ps_pj:

            l_pad = pd.tile([128, QW], BF16, tag="l_pad", bufs=1, name="l_pad")
            nc.vector.memset(l_pad[:, :], 0.0)
            # two alternating sT tiles (separate tensors -> independent WAR
            # chains; a single tile serializes every score matmul behind the
            # immediately preceding exp because reads are tracked per-tile)
            sTs = [ps_sT.tile([128, 1024], F32, tag=f"sT{i}", name=f"sT{i}", bufs=1)
                   for i in range(2)]
            gkp = [0]

            LAG = 3  # outT matmuls run LAG kp-iterations behind sT/exp
            pending_norm = []

            def attn(h, qc):
                fb, hh = divmod(h, 2)
                q_ap = qT[fb][:, qc * QW:(qc + 1) * QW]
                outT = ps_oT.tile([65, QW], F32, tag="outT")
                NKP = NT // 2
                pTs = {}

                def emit_outT(kp):
                    for j in range(2):
                        kt = kp * 2 + j
                        nc.tensor.matmul(
                            outT[:, :],
                            v_ext[kt][:, h * 65:(h + 1) * 65],
                            pTs[kp][:, j * 512:(j + 1) * 512],
                            start=(kp == 0 and j == 0),
                            stop=(kp == NKP - 1 and j == 1))

                for kp in range(NKP):
                    sT = sTs[gkp[0] % 2]
                    gkp[0] += 1
                    for j in range(2):
                        kt = kp * 2 + j
                        nc.tensor.matmul(
                            sT[:, j * 512:(j + 1) * 512],
                            kTh[h][:, kt * 128:(kt + 1) * 128],
                            q_ap, start=True, stop=True)
                    pT = pd.tile([128, 1024], BF16, tag="pT", bufs=5)
                    nc.scalar.activation(pT[:, :], sT[:, :], AF.Exp)
                    pTs[kp] = pT
                    if kp == 1 and pending_norm:
                        # previous head's normalization: inputs long since
                        # ready; emitting here keeps the PE stream stall-free
                        pending_norm.pop(0)()
                    if kp >= LAG:
                        emit_outT(kp - LAG)
                        del pTs[kp - LAG]
                for kp in range(NKP - LAG, NKP):
                    emit_outT(kp)

                def norm():
                    # broadcast denominators l across 64 partitions via the
                    # one-hot-row matmul, then a partition-parallel reciprocal
                    # (a [1,512] DVE op runs on one lane = ~3.4us; avoid it)
                    nc.scalar.copy(l_pad[0:1, :], outT[64:65, :])
                    rb_ps = ps_rb.tile([64, QW], F32, tag="rb")
                    nc.tensor.matmul(rb_ps[:, :], Emat[:, :], l_pad[:, :],
                                     start=True, stop=True)
                    rb = pd.tile([64, QW], F32, tag="rb_sb")
                    nc.vector.reciprocal(rb[:, :], rb_ps[:, :])
                    nc.vector.tensor_mul(
                        lout[fb][hh * 64:(hh + 1) * 64, qc * QW:(qc + 1) * QW],
                        outT[0:64, :], rb[:, :])
                pending_norm.append(norm)

            rs_mode = os.environ.get("KERNEL_RS_MODE", "one")
            if rs_mode == "one":
                rs_in_big = pdram.tile([T, C], BF16, tag="rs_in_big")
                rs_out_big = pdram.tile([T // 2, C], BF16, tag="rs_out_big")
            rs_chunks_done = []

            def proj_chunk(tq, rs_in, row_base):
                for half in range(2):
                    for t2 in range(2):
                        tok0 = half * 1024 + tq * 256 + t2 * 128
                        for cc in range(2):
                            pj = ps_pj.tile([128, 512], F32, tag="pj")
                            for fb in range(NFB):
                                nc.tensor.matmul(
                                    pj[:, :],
                                    lout[fb][:, tok0:tok0 + 128],
                                    woT[fb][:, cc * 512:(cc + 1) * 512],
                                    start=(fb == 0), stop=(fb == NFB - 1))
                            ot = pd.tile([128, 512], BF16, tag="ot")
                            nc.vector.tensor_add(
                                ot[:, :], pj[:, :],
                                bo_bcast[:, cc * 512:(cc + 1) * 512])
                            r0 = row_base(half) + t2 * 128
                            nc.sync.dma_start(
                                rs_in[r0:r0 + 128, cc * 512:(cc + 1) * 512],
                                ot[:, :])

            def drain_rows(rs_out, src_row0, out_row0, nrows):
                for t2 in range(nrows // 128):
                    fo_bf = pd.tile([128, C], BF16, tag="fo_bf")
                    nc.sync.dma_start(
                        fo_bf[:, :],
                        rs_out[src_row0 + t2 * 128: src_row0 + (t2 + 1) * 128, :])
                    fo = pd.tile([128, C], F32, tag="fo")
                    nc.vector.tensor_copy(fo[:, :], fo_bf[:, :])
                    nc.sync.dma_start(
                        out_ext[out_row0 + t2 * 128: out_row0 + (t2 + 1) * 128, :],
                        fo[:, :])

            def proj_rs(tq):
                phases = os.environ.get("KERNEL_PHASES", "full")
                if rs_mode == "one":
                    proj_chunk(tq, rs_in_big,
                               lambda half: half * 1024 + tq * 256)
                    rs_chunks_done.append(tq)
                    if len(rs_chunks_done) == 4:
                        if phases == "nors":
                            nc.sync.dma_start(rs_out_big[:, :], rs_in_big[0:T // 2, :])
                        else:
                            nc.gpsimd.collective_compute(
                                "ReduceScatter", mybir.AluOpType.add,
                                replica_groups=PAIRS,
                                ins=[rs_in_big.opt()], outs=[rs_out_big.opt()])
                        drain_rows(rs_out_big, 0, 0, T // 2)
                else:
                    rs_in = pdram.tile([512, C], BF16, tag="rs_in", name="rs_in")
                    rs_out = pdram.tile([256, C], BF16, tag="rs_out", name="rs_out")
                    proj_chunk(tq, rs_in, lambda half: half * 256)
                    if phases == "nors":
                        nc.sync.dma_start(rs_out[:, :], rs_in[0:256, :])
                    else:
                        nc.gpsimd.collective_compute(
                            "ReduceScatter", mybir.AluOpType.add,
                            replica_groups=PAIRS,
                            ins=[rs_in.opt()], outs=[rs_out.opt()])
                    drain_rows(rs_out, 0, tq * 256, 256)

            phases = os.environ.get("KERNEL_PHASES", "full")
            if phases == "qkv":
                dbg = pd.tile([128, C], F32, tag="dbg")
                nc.vector.tensor_copy(dbg[:, :], qT[0][:, 0:1024])
                nc.sync.dma_start(out_ext[0:128, :], dbg[:, :])
            else:
                for pair_i, (qca, qcb) in enumerate(((0, 2), (1, 3))):
                    for qc in (qca, qcb):
                        for fb in range(NFB):
                            for hh in range(2):
                                attn(fb * 2 + hh, qc)
                    while pending_norm:
                        pending_norm.pop(0)()
                    if phases != "attn":
                        for tq in (pair_i * 2, pair_i * 2 + 1):
                            proj_rs(tq)
                if phases == "attn":
                    dbg = pd.tile([128, C], F32, tag="dbg")
                    for fb in range(NFB):
                        nc.vector.tensor_copy(dbg[:, :], lout[fb][:, 0:1024])
                        nc.sync.dma_start(out_ext[fb * 128:(fb + 1) * 128, :], dbg[:, :])


def _build_nc():
    nc = bacc.Bacc("TRN2", target_bir_lowering=False, debug=False,
                   num_devices=N_CORES)
    x_ext = nc.dram_tensor("x", [T, C], BF16, kind="ExternalInput")
    wqt_ext = nc.dram_tensor("wqt", [C, FS], BF16, kind="ExternalInput")
    wkt_ext = nc.dram_tensor("wkt", [C, FS], BF16, kind="ExternalInput")
    wvt_ext = nc.dram_tensor("wvt", [C, FS], BF16, kind="ExternalInput")
    wot_ext = nc.dram_tensor("wot", [FS, C], BF16, kind="ExternalInput")
    bo_ext = nc.dram_tensor("bo", [C], F32, kind="ExternalInput")
    out_ext = nc.dram_tensor("out", [T // 2, C], F32, kind="ExternalOutput")
    with tile.TileContext(nc) as tc:
        _emit(nc, tc, x_ext, wqt_ext, wkt_ext, wvt_ext, wot_ext, bo_ext, out_ext)
    nc.finalize()
    return nc


# ---------------------------------------------------------------------------
# NTFF profiling under axon (used when KERNEL_TRACE=1): the agent image's
# antenv lacks axon_hooks, so inject an equivalent module backed by the
# libaxon_pjrt.so profiling C ABI.
# ---------------------------------------------------------------------------
def _ensure_axon_hooks():
    try:
        from antenv.axon_hooks import get_axon_ntff_profile_hook  # noqa: F401
        return
    except ImportError:
        pass
    import ctypes
    import antenv

    so_path = "/opt/axon/libaxon_pjrt.so"
    lib = ctypes.CDLL(so_path)
    if not hasattr(lib, "axon_start_nrt_profile"):
        return
    lib.axon_start_nrt_profile.argtypes = [ctypes.POINTER(ctypes.c_int64),
                                           ctypes.c_size_t]
    lib.axon_start_nrt_profile.restype = ctypes.c_int64
    lib.axon_stop_nrt_profile.argtypes = [ctypes.c_char_p]
    lib.axon_stop_nrt_profile.restype = ctypes.c_int64

    @contextlib.contextmanager
    def _hook(output_dir, device_ids):
        import jax
        jax.devices()
        if device_ids:
            ids = (ctypes.c_int64 * len(device_ids))(*device_ids)
            rc = lib.axon_start_nrt_profile(ids, len(device_ids))
        else:
            rc = lib.axon_start_nrt_profile(None, 0)
        if rc != 0:
            raise RuntimeError(f"axon_start_nrt_profile rc={rc}")
        try:
            yield
        finally:
            n = lib.axon_stop_nrt_profile(str(output_dir).encode())
            print(f"ntff profile: {n} file(s) -> {output_dir}", file=sys.stderr)

    holder = [_hook]
    mod = types.ModuleType("antenv.axon_hooks")
    mod.get_axon_ntff_profile_hook = lambda: holder[0]
    mod.set_axon_ntff_profile_hook = lambda h: holder.__setitem__(0, h)
    sys.modules["antenv.axon_hooks"] = mod
    antenv.axon_hooks = mod
    # avoid S3 upload attempts during profile post-processing
    bass_utils.upload_artifacts = lambda tmpdir: f"(local:{tmpdir})"


_NC = None
LAST = {}


def kernel(hidden_states, wq, wk, wv, wo, bo):
    global _NC
    hidden_states = np.asarray(hidden_states, dtype=np.float32)
    wq = np.asarray(wq, dtype=np.float32)
    wk = np.asarray(wk, dtype=np.float32)
    wv = np.asarray(wv, dtype=np.float32)
    wo = np.asarray(wo, dtype=np.float32)
    bo = np.asarray(bo, dtype=np.float32)

    if _NC is None:
        _NC = _build_nc()

    bf = ml_dtypes.bfloat16
    scale = np.float32(D ** -0.5)
    in_maps = []
    for c in range(N_CORES):
        b, hg = divmod(c, 2)
        fr = hg * FS
        in_maps.append({
            "x": np.ascontiguousarray(hidden_states[b]).astype(bf),
            "wqt": np.ascontiguousarray((wq[fr:fr + FS] * scale).T).astype(bf),
            "wkt": np.ascontiguousarray(wk[fr:fr + FS].T).astype(bf),
            "wvt": np.ascontiguousarray(wv[fr:fr + FS].T).astype(bf),
            "wot": np.ascontiguousarray(wo[:, fr:fr + FS].T).astype(bf),
            "bo": bo * np.float32(0.5),
        })

    trace = os.environ.get("KERNEL_TRACE", "0") == "1"
    if trace:
        _ensure_axon_hooks()
    res = bass_utils.run_bass_kernel_spmd(
        _NC, in_maps, core_ids=list(range(N_CORES)), trace=trace)
    LAST["exec_time_ns"] = res.exec_time_ns
    LAST["res"] = res

    y = np.empty((B, T, C), dtype=np.float32)
    for c in range(N_CORES):
        b, hg = divmod(c, 2)
        y[b, hg * (T // 2):(hg + 1) * (T // 2), :] = res.results[c]["out"]
    return y


# revision 14
# speedup vs baseline: 1.4937x; 1.0843x over previous
"""Trainium2 Bass kernel for nn_Attention_84567906058480.

Multi-head attention (B=4, T=2048, C=1024, H=16, D=64) on 8 NeuronCores.

Sharding: core c = (batch b = c//2, head-group hg = c%2).  Each core computes
Q/K/V for its 8 heads over its batch (tensor-parallel split of wq/wk/wv rows),
runs attention, applies its column-slice of wo to get a partial output, and a
pairwise ReduceScatter (groups [2b, 2b+1]) sums the two head-group partials
while scattering token halves: the even core ends with tokens [0,1024) of its
batch, the odd core with tokens [1024,2048).  The host concatenates.

Implementation notes:
- Activations/weights run bf16 on the PE (f32 PSUM accumulate); rel-err ~5e-3.
- Weights are pre-transposed (and wq pre-scaled by 1/sqrt(D)) on the host and
  shipped bf16, so only x needs on-chip transposes.
- Scores are computed directly transposed (S.T = k.T-tiles @ qT) so no
  P-transpose is needed; the softmax denominator comes from a ones-column
  appended to V (M=65 stationary operand); exp needs no max-subtraction
  (|scores| < ~3 by construction).
- Every attention matmul contracts over K=128 (per-head K tensors are
  zero-padded into the other head's partition range) so the PE array never
  switches tiling modes; score and output matmul emission is software-
  pipelined (outputs lag scores by 2 iterations, normalization is deferred
  past the next head's start) to keep the PE stream stall-free.
- The output bias bo is halved on the host so the pairwise reduce adds it
  exactly once.
"""

import os
import sys
import types
import contextlib

import numpy as np

if "/opt/trn_rl_repo" not in sys.path:
    sys.path.insert(0, "/opt/trn_rl_repo")

import ml_dtypes
import concourse.bass as bass  # noqa: F401
import concourse.mybir as mybir
import concourse.tile as tile
from concourse import bacc
from concourse import bass_utils
from concourse.masks import make_identity

F32 = mybir.dt.float32
BF16 = mybir.dt.bfloat16
AF = mybir.ActivationFunctionType

B, T, C = 4, 2048, 1024
H, D = 16, 64
HPC = 8            # heads per core
FS = HPC * D       # per-core feature shard = 512
N_CORES = 8
PAIRS = [[0, 1], [2, 3], [4, 5], [6, 7]]

NT = T // 128      # 16 token tiles
NCT = C // 128     # 8 contraction tiles
NFB = FS // 128    # 4 feature blocks per core
QW = 512           # q chunk width
NQC = T // QW      # 4 q chunks


def _emit(nc, tc, x_ext, wqt_ext, wkt_ext, wvt_ext, wot_ext, bo_ext, out_ext):
    with tc.tile_pool(name="const", bufs=1) as constp, \
         tc.tile_pool(name="persist", bufs=1) as pp:

        # ---- constants -------------------------------------------------
        identb = constp.tile([128, 128], BF16, tag="identb")
        make_identity(nc, identb[:, :])
        ones_col = constp.tile([1, 128], F32, tag="ones")
        nc.gpsimd.memset(ones_col[:, :], 1.0)
        Emat = constp.tile([128, 64], BF16, tag="Emat")
        nc.gpsimd.memset(Emat[:, :], 0.0)
        nc.gpsimd.memset(Emat[0:1, :], 1.0)
        bo_row = constp.tile([1, C], F32, tag="bo_row")
        nc.sync.dma_start(bo_row[:, :], bo_ext[:].unsqueeze(0))
        bo_bcast = constp.tile([128, C], F32, tag="bo_bcast")

        # ---- persistent activation storage (bf16) ----------------------
        qT = [pp.tile([128, T], BF16, tag=f"qT{fb}", name=f"qT{fb}") for fb in range(NFB)]
        kTh = [pp.tile([128, T], BF16, tag=f"kTh{h}", name=f"kTh{h}") for h in range(HPC)]
        v_ext = [pp.tile([128, HPC * 65], BF16, tag=f"vx{tt}", name=f"vx{tt}") for tt in range(NT)]
        woT = [pp.tile([128, C], BF16, tag=f"woT{fb}", name=f"woT{fb}") for fb in range(NFB)]
        lout = [pp.tile([128, T], BF16, tag=f"lo{fb}", name=f"lo{fb}") for fb in range(NFB)]

        # =================================================================
        # Phase B/C: weight loads, x transposes, QKV projections
        # =================================================================
        with tc.tile_pool(name="pbc", bufs=2) as pbc, \
             tc.tile_pool(name="ps_tr", bufs=4, space="PSUM") as ps_tr, \
             tc.tile_pool(name="ps_acc", bufs=2, space="PSUM") as ps_acc:

            # bias broadcast [128, C] via rank-1 ones matmul (exact f32)
            for cc in range(2):
                bb = ps_acc.tile([128, 512], F32, tag="acc")
                nc.tensor.matmul(bb[:, :], ones_col[:, :],
                                 bo_row[:, cc * 512:(cc + 1) * 512],
                                 start=True, stop=True)
                nc.vector.tensor_copy(bo_bcast[:, cc * 512:(cc + 1) * 512], bb[:, :])

            # ---- weights: direct (host-pre-transposed) loads ------------
            def ctile_major(ext):
                return ext[:].rearrange("(ct p) f -> p ct f", p=128)

            # kTh[h]: head h's k at partitions (h%2)*64..+64, zeros in the
            # other half -> K=128 score matmuls with the full-qT rhs.
            for h in range(HPC):
                z0 = (1 - (h % 2)) * 64
                nc.gpsimd.memset(kTh[h][z0:z0 + 64, :], 0.0)

            # ---- xT: transpose x (bf16) into [C-part, tok] --------------
            # weight DMAs are interleaved after the first x tiles so the PE
            # gets transpose work immediately
            xT = [pbc.tile([128, T], BF16, tag=f"xT{ct}", name=f"xT{ct}", bufs=1) for ct in range(NCT)]
            wqTf = pbc.tile([128, NCT * FS], BF16, tag="wqTf", bufs=1)
            wkTf = pbc.tile([128, NCT * FS], BF16, tag="wkTf", bufs=1)
            wvT = pbc.tile([128, NCT * FS], BF16, tag="wvT", bufs=1)
            for tt in range(NT):
                xnat = pbc.tile([128, C], BF16, tag="xnat", bufs=3)
                nc.sync.dma_start(xnat[:, :], x_ext[tt * 128:(tt + 1) * 128, :])
                if tt == 2:
                    nc.sync.dma_start(wvT[:].rearrange("p (ct f) -> p ct f", f=FS),
                                      ctile_major(wvt_ext))
                elif tt == 4:
                    nc.sync.dma_start(wqTf[:].rearrange("p (ct f) -> p ct f", f=FS),
                                      ctile_major(wqt_ext))
                elif tt == 6:
                    nc.sync.dma_start(wkTf[:].rearrange("p (ct f) -> p ct f", f=FS),
                                      ctile_major(wkt_ext))
                elif tt == 8:
                    for fb in range(NFB):
                        nc.sync.dma_start(woT[fb][:, :],
                                          wot_ext[fb * 128:(fb + 1) * 128, :])
                for ct in range(NCT):
                    tr = ps_tr.tile([128, 128], BF16, tag="tr")
                    nc.tensor.transpose(tr[:, :], xnat[:, ct * 128:(ct + 1) * 128],
                                        identb[:, :])
                    nc.vector.tensor_copy(xT[ct][:, tt * 128:(tt + 1) * 128], tr[:, :])

            # ---- q/k projections ---------------------------------------
            for fb in range(NFB):
                for name, wf in (("wq", wqTf), ("wk", wkTf)):
                    for tch in range(NQC):
                        acc = ps_acc.tile([128, QW], F32, tag="acc")
                        for ct in range(NCT):
                            nc.tensor.matmul(
                                acc[:, :],
                                wf[:, ct * FS + fb * 128: ct * FS + fb * 128 + 128],
                                xT[ct][:, tch * QW:(tch + 1) * QW],
                                start=(ct == 0), stop=(ct == NCT - 1))
                        if name == "wq":
                            nc.vector.tensor_copy(
                                qT[fb][:, tch * QW:(tch + 1) * QW], acc[:, :])
                        else:
                            for hh in range(2):
                                nc.vector.tensor_copy(
                                    kTh[fb * 2 + hh][hh * 64:(hh + 1) * 64,
                                                     tch * QW:(tch + 1) * QW],
                                    acc[hh * 64:(hh + 1) * 64, :])

            # ---- v: natural [tok, feat] with ones column interleave -----
            for tt in range(NT):
                acc = ps_acc.tile([128, FS], F32, tag="acc")
                for ct in range(NCT):
                    nc.tensor.matmul(
                        acc[:, :],
                        xT[ct][:, tt * 128:(tt + 1) * 128],
                        wvT[:, ct * FS:(ct + 1) * FS],
                        start=(ct == 0), stop=(ct == NCT - 1))
                nc.gpsimd.memset(v_ext[tt][:, :], 1.0)
                dst = v_ext[tt][:].rearrange("p (h e) -> p h e", e=65)[:, :, 0:64]
                src = acc[:].rearrange("p (h e) -> p h e", e=64)
                nc.vector.tensor_copy(dst, src)

        # =================================================================
        # Phase D/E: attention + output projection + ReduceScatter
        # =================================================================
        with tc.tile_pool(name="pd", bufs=4) as pd, \
             tc.tile_pool(name="pdram", bufs=4, space="DRAM") as pdram, \
             tc.tile_pool(name="ps_sT", bufs=1, space="PSUM") as ps_sT, \
             tc.tile_pool(name="ps_oT", bufs=2, space="PSUM") as ps_oT, \
             tc.tile_pool(name="ps_rb", bufs=1, space="PSUM") as ps_rb, \
             tc.tile_pool(name="ps_pj", bufs=1, space="PSUM") as ps_pj:

            l_pad = pd.tile([128, QW], BF16, tag="l_pad", bufs=1, name="l_pad")
            nc.gpsimd.memset(l_pad[:, :], 0.0)
            # two alternating sT tiles (separate tensors -> independent WAR
            # chains; a single tile serializes every score matmul behind the
            # immediately preceding exp because reads are tracked per-tile)
            sTs = [ps_sT.tile([128, 1024], F32, tag=f"sT{i}", name=f"sT{i}", bufs=1)
                   for i in range(2)]
            gkp = [0]

            LAG = 3  # outT matmuls run LAG kp-iterations behind sT/exp
            pending_norm = []

            def attn(h, qc):
                fb, hh = divmod(h, 2)
                q_ap = qT[fb][:, qc * QW:(qc + 1) * QW]
                outT = ps_oT.tile([65, QW], F32, tag="outT")
                NKP = NT // 2
                pTs = {}

                def emit_outT(kp):
                    for j in range(2):
                        kt = kp * 2 + j
                        nc.tensor.matmul(
                            outT[:, :],
                            v_ext[kt][:, h * 65:(h + 1) * 65],
                            pTs[kp][:, j * 512:(j + 1) * 512],
                            start=(kp == 0 and j == 0),
                            stop=(kp == NKP - 1 and j == 1))

                for kp in range(NKP):
                    sT = sTs[gkp[0] % 2]
                    gkp[0] += 1
                    for j in range(2):
                        kt = kp * 2 + j
                        nc.tensor.matmul(
                            sT[:, j * 512:(j + 1) * 512],
                            kTh[h][:, kt * 128:(kt + 1) * 128],
                            q_ap, start=True, stop=True)
                    pT = pd.tile([128, 1024], BF16, tag="pT", bufs=5)
                    nc.scalar.activation(pT[:, :], sT[:, :], AF.Exp)
                    pTs[kp] = pT
                    if kp == 1 and pending_norm:
                        # previous head's normalization: inputs long since
                        # ready; emitting here keeps the PE stream stall-free
                        pending_norm.pop(0)()
                    if kp >= LAG:
                        emit_outT(kp - LAG)
                        del pTs[kp - LAG]
                for kp in range(NKP - LAG, NKP):
                    emit_outT(kp)

                def norm():
                    # broadcast denominators l across 64 partitions via the
                    # one-hot-row matmul, then a partition-parallel reciprocal
                    # (a [1,512] DVE op runs on one lane = ~3.4us; avoid it)
                    nc.scalar.copy(l_pad[0:1, :], outT[64:65, :])
                    rb_ps = ps_rb.tile([64, QW], F32, tag="rb")
                    nc.tensor.matmul(rb_ps[:, :], Emat[:, :], l_pad[:, :],
                                     start=True, stop=True)
                    rb = pd.tile([64, QW], F32, tag="rb_sb")
                    nc.vector.reciprocal(rb[:, :], rb_ps[:, :])
                    nc.vector.tensor_mul(
                        lout[fb][hh * 64:(hh + 1) * 64, qc * QW:(qc + 1) * QW],
                        outT[0:64, :], rb[:, :])
                pending_norm.append(norm)

            rs_mode = os.environ.get("KERNEL_RS_MODE", "two")
            if rs_mode == "one":
                rs_in_big = pdram.tile([T, C], BF16, tag="rs_in_big")
                rs_out_big = pdram.tile([T // 2, C], BF16, tag="rs_out_big")
            rs_chunks_done = []
            rs2_state = {}

            def proj_chunk(tq, rs_in, row_base):
                for half in range(2):
                    for t2 in range(2):
                        tok0 = half * 1024 + tq * 256 + t2 * 128
                        for cc in range(2):
                            pj = ps_pj.tile([128, 512], F32, tag="pj")
                            for fb in range(NFB):
                                nc.tensor.matmul(
                                    pj[:, :],
                                    lout[fb][:, tok0:tok0 + 128],
                                    woT[fb][:, cc * 512:(cc + 1) * 512],
                                    start=(fb == 0), stop=(fb == NFB - 1))
                            ot = pd.tile([128, 512], BF16, tag="ot")
                            nc.vector.tensor_add(
                                ot[:, :], pj[:, :],
                                bo_bcast[:, cc * 512:(cc + 1) * 512])
                            r0 = row_base(half) + t2 * 128
                            nc.sync.dma_start(
                                rs_in[r0:r0 + 128, cc * 512:(cc + 1) * 512],
                                ot[:, :])

            def drain_rows(rs_out, src_row0, out_row0, nrows):
                for t2 in range(nrows // 128):
                    fo_bf = pd.tile([128, C], BF16, tag="fo_bf")
                    nc.sync.dma_start(
                        fo_bf[:, :],
                        rs_out[src_row0 + t2 * 128: src_row0 + (t2 + 1) * 128, :])
                    fo = pd.tile([128, C], F32, tag="fo")
                    nc.vector.tensor_copy(fo[:, :], fo_bf[:, :])
                    nc.sync.dma_start(
                        out_ext[out_row0 + t2 * 128: out_row0 + (t2 + 1) * 128, :],
                        fo[:, :])

            def proj_rs(tq):
                phases = os.environ.get("KERNEL_PHASES", "full")
                if rs_mode == "one":
                    proj_chunk(tq, rs_in_big,
                               lambda half: half * 1024 + tq * 256)
                    rs_chunks_done.append(tq)
                    if len(rs_chunks_done) == 4:
                        if phases == "nors":
                            nc.sync.dma_start(rs_out_big[:, :], rs_in_big[0:T // 2, :])
                        else:
                            nc.gpsimd.collective_compute(
                                "ReduceScatter", mybir.AluOpType.add,
                                replica_groups=PAIRS,
                                ins=[rs_in_big.opt()], outs=[rs_out_big.opt()])
                        drain_rows(rs_out_big, 0, 0, T // 2)
                elif rs_mode == "two":
                    # chunk i covers q-chunk pair (i, i+2): rows 0-511 =
                    # half0 tokens i*512..+512, rows 512-1023 = half1 same
                    i = tq // 2
                    if tq % 2 == 0:
                        self_rs = pdram.tile([1024, C], BF16, tag="rs_in2",
                                             name=f"rs_in2_{i}")
                        rs2_state[i] = self_rs
                    rs_in = rs2_state[i]
                    proj_chunk(tq, rs_in,
                               lambda half: half * 512 + (tq % 2) * 256)
                    if tq % 2 == 1:
                        rs_out = pdram.tile([512, C], BF16, tag="rs_out2",
                                            name=f"rs_out2_{i}")
                        if phases == "nors":
                            nc.sync.dma_start(rs_out[:, :], rs_in[0:512, :])
                        else:
                            nc.gpsimd.collective_compute(
                                "ReduceScatter", mybir.AluOpType.add,
                                replica_groups=PAIRS,
                                ins=[rs_in.opt()], outs=[rs_out.opt()])
                        drain_rows(rs_out, 0, i * 512, 512)
                else:
                    rs_in = pdram.tile([512, C], BF16, tag="rs_in", name="rs_in")
                    rs_out = pdram.tile([256, C], BF16, tag="rs_out", name="rs_out")
                    proj_chunk(tq, rs_in, lambda half: half * 256)
                    if phases == "nors":
                        nc.sync.dma_start(rs_out[:, :], rs_in[0:256, :])
                    else:
                        nc.gpsimd.collective_compute(
                            "ReduceScatter", mybir.AluOpType.add,
                            replica_groups=PAIRS,
                            ins=[rs_in.opt()], outs=[rs_out.opt()])
                    drain_rows(rs_out, 0, tq * 256, 256)

            phases = os.environ.get("KERNEL_PHASES", "full")
            if phases == "qkv":
                dbg = pd.tile([128, C], F32, tag="dbg")
                nc.vector.tensor_copy(dbg[:, :], qT[0][:, 0:1024])
                nc.sync.dma_start(out_ext[0:128, :], dbg[:, :])
            else:
                for pair_i, (qca, qcb) in enumerate(((0, 2), (1, 3))):
                    for qc in (qca, qcb):
                        for fb in range(NFB):
                            for hh in range(2):
                                attn(fb * 2 + hh, qc)
                    while pending_norm:
                        pending_norm.pop(0)()
                    if phases != "attn":
                        for tq in (pair_i * 2, pair_i * 2 + 1):
                            proj_rs(tq)
                if phases == "attn":
                    dbg = pd.tile([128, C], F32, tag="dbg")
                    for fb in range(NFB):
                        nc.vector.tensor_copy(dbg[:, :], lout[fb][:, 0:1024])
                        nc.sync.dma_start(out_ext[fb * 128:(fb + 1) * 128, :], dbg[:, :])


def _build_nc():
    nc = bacc.Bacc("TRN2", target_bir_lowering=False, debug=False,
                   num_devices=N_CORES)
    x_ext = nc.dram_tensor("x", [T, C], BF16, kind="ExternalInput")
    wqt_ext = nc.dram_tensor("wqt", [C, FS], BF16, kind="ExternalInput")
    wkt_ext = nc.dram_tensor("wkt", [C, FS], BF16, kind="ExternalInput")
    wvt_ext = nc.dram_tensor("wvt", [C, FS], BF16, kind="ExternalInput")
    wot_ext = nc.dram_tensor("wot", [FS, C], BF16, kind="ExternalInput")
    bo_ext = nc.dram_tensor("bo", [C], F32, kind="ExternalInput")
    out_ext = nc.dram_tensor("out", [T // 2, C], F32, kind="ExternalOutput")
    with tile.TileContext(nc) as tc:
        _emit(nc, tc, x_ext, wqt_ext, wkt_ext, wvt_ext, wot_ext, bo_ext, out_ext)
    nc.finalize()
    return nc


# ---------------------------------------------------------------------------
# NTFF profiling under axon (used when KERNEL_TRACE=1): the agent image's
# antenv lacks axon_hooks, so inject an equivalent module backed by the
# libaxon_pjrt.so profiling C ABI.
# ---------------------------------------------------------------------------
def _ensure_axon_hooks():
    try:
        from antenv.axon_hooks import get_axon_ntff_profile_hook  # noqa: F401
        return
    except ImportError:
        pass
    import ctypes
    import antenv

    so_path = "/opt/axon/libaxon_pjrt.so"
    lib = ctypes.CDLL(so_path)
    if not hasattr(lib, "axon_start_nrt_profile"):
        return
    lib.axon_start_nrt_profile.argtypes = [ctypes.POINTER(ctypes.c_int64),
                                           ctypes.c_size_t]
    lib.axon_start_nrt_profile.restype = ctypes.c_int64
    lib.axon_stop_nrt_profile.argtypes = [ctypes.c_char_p]
    lib.axon_stop_nrt_profile.restype = ctypes.c_int64

    @contextlib.contextmanager
    def _hook(output_dir, device_ids):
        import jax
        jax.devices()
        if device_ids:
            ids = (ctypes.c_int64 * len(device_ids))(*device_ids)
            rc = lib.axon_start_nrt_profile(ids, len(device_ids))
        else:
            rc = lib.axon_start_nrt_profile(None, 0)
        if rc != 0:
            raise RuntimeError(f"axon_start_nrt_profile rc={rc}")
        try:
            yield
        finally:
            n = lib.axon_stop_nrt_profile(str(output_dir).encode())
            print(f"ntff profile: {n} file(s) -> {output_dir}", file=sys.stderr)

    holder = [_hook]
    mod = types.ModuleType("antenv.axon_hooks")
    mod.get_axon_ntff_profile_hook = lambda: holder[0]
    mod.set_axon_ntff_profile_hook = lambda h: holder.__setitem__(0, h)
    sys.modules["antenv.axon_hooks"] = mod
    antenv.axon_hooks = mod
    # avoid S3 upload attempts during profile post-processing
    bass_utils.upload_artifacts = lambda tmpdir: f"(local:{tmpdir})"


_NC = None
LAST = {}


def kernel(hidden_states, wq, wk, wv, wo, bo):
    global _NC
    hidden_states = np.asarray(hidden_states, dtype=np.float32)
    wq = np.asarray(wq, dtype=np.float32)
    wk = np.asarray(wk, dtype=np.float32)
    wv = np.asarray(wv, dtype=np.float32)
    wo = np.asarray(wo, dtype=np.float32)
    bo = np.asarray(bo, dtype=np.float32)

    if _NC is None:
        _NC = _build_nc()

    bf = ml_dtypes.bfloat16
    scale = np.float32(D ** -0.5)
    in_maps = []
    for c in range(N_CORES):
        b, hg = divmod(c, 2)
        fr = hg * FS
        in_maps.append({
            "x": np.ascontiguousarray(hidden_states[b]).astype(bf),
            "wqt": np.ascontiguousarray((wq[fr:fr + FS] * scale).T).astype(bf),
            "wkt": np.ascontiguousarray(wk[fr:fr + FS].T).astype(bf),
            "wvt": np.ascontiguousarray(wv[fr:fr + FS].T).astype(bf),
            "wot": np.ascontiguousarray(wo[:, fr:fr + FS].T).astype(bf),
            "bo": bo * np.float32(0.5),
        })

    trace = os.environ.get("KERNEL_TRACE", "0") == "1"
    if trace:
        _ensure_axon_hooks()
    res = bass_utils.run_bass_kernel_spmd(
        _NC, in_maps, core_ids=list(range(N_CORES)), trace=trace)
    LAST["exec_time_ns"] = res.exec_time_ns
    LAST["res"] = res

    y = np.empty((B, T, C), dtype=np.float32)
    for c in range(N_CORES):
        b, hg = divmod(c, 2)
        y[b, hg * (T // 2):(hg + 1) * (T // 2), :] = res.results[c]["out"]
    return y


# revision 15
# speedup vs baseline: 1.6867x; 1.1293x over previous
"""Trainium2 Bass kernel for nn_Attention_84567906058480.

Multi-head attention (B=4, T=2048, C=1024, H=16, D=64) on 8 NeuronCores.

Sharding: core c = (batch b = c//2, head-group hg = c%2).  Each core computes
Q/K/V for its 8 heads over its batch (tensor-parallel split of wq/wk/wv rows),
runs attention, applies its column-slice of wo to get a partial output, and a
pairwise ReduceScatter (groups [2b, 2b+1]) sums the two head-group partials
while scattering token halves: the even core ends with tokens [0,1024) of its
batch, the odd core with tokens [1024,2048).  The host concatenates.

Implementation notes:
- Activations/weights run bf16 on the PE (f32 PSUM accumulate); rel-err ~5e-3.
- Weights are pre-transposed (and wq pre-scaled by 1/sqrt(D)) on the host and
  shipped bf16, so only x needs on-chip transposes.
- Scores are computed directly transposed (S.T = k.T-tiles @ qT) so no
  P-transpose is needed; the softmax denominator comes from a ones-column
  appended to V (M=65 stationary operand); exp needs no max-subtraction
  (|scores| < ~3 by construction).
- Every attention matmul contracts over K=128 (per-head K tensors are
  zero-padded into the other head's partition range) so the PE array never
  switches tiling modes; score and output matmul emission is software-
  pipelined (outputs lag scores by 2 iterations, normalization is deferred
  past the next head's start) to keep the PE stream stall-free.
- The output bias bo is halved on the host so the pairwise reduce adds it
  exactly once.
"""

import os
import sys
import types
import contextlib

import numpy as np

if "/opt/trn_rl_repo" not in sys.path:
    sys.path.insert(0, "/opt/trn_rl_repo")

import ml_dtypes
import concourse.bass as bass  # noqa: F401
import concourse.mybir as mybir
import concourse.tile as tile
from concourse import bacc
from concourse import bass_utils
from concourse.masks import make_identity

F32 = mybir.dt.float32
BF16 = mybir.dt.bfloat16
AF = mybir.ActivationFunctionType

B, T, C = 4, 2048, 1024
H, D = 16, 64
HPC = 8            # heads per core
FS = HPC * D       # per-core feature shard = 512
N_CORES = 8
PAIRS = [[0, 1], [2, 3], [4, 5], [6, 7]]

NT = T // 128      # 16 token tiles
NCT = C // 128     # 8 contraction tiles
NFB = FS // 128    # 4 feature blocks per core
QW = 512           # q chunk width
NQC = T // QW      # 4 q chunks


def _emit(nc, tc, x_ext, wqt_ext, wkt_ext, wvt_ext, wot_ext, bo_ext, out_ext):
    with tc.tile_pool(name="const", bufs=1) as constp, \
         tc.tile_pool(name="persist", bufs=1) as pp:

        # ---- constants -------------------------------------------------
        identb = constp.tile([128, 128], BF16, tag="identb")
        make_identity(nc, identb[:, :])
        ones_col = constp.tile([1, 128], F32, tag="ones")
        nc.gpsimd.memset(ones_col[:, :], 1.0)
        Emat = constp.tile([128, 64], BF16, tag="Emat")
        nc.gpsimd.memset(Emat[:, :], 0.0)
        nc.gpsimd.memset(Emat[0:1, :], 1.0)
        bo_row = constp.tile([1, C], F32, tag="bo_row")
        nc.sync.dma_start(bo_row[:, :], bo_ext[:].unsqueeze(0))
        bo_bcast = constp.tile([128, C], F32, tag="bo_bcast")

        # ---- persistent activation storage (bf16) ----------------------
        qT = [pp.tile([128, T], BF16, tag=f"qT{fb}", name=f"qT{fb}") for fb in range(NFB)]
        kTh = [pp.tile([128, T], BF16, tag=f"kTh{h}", name=f"kTh{h}") for h in range(HPC)]
        v_ext = [pp.tile([128, HPC * 65], BF16, tag=f"vx{tt}", name=f"vx{tt}") for tt in range(NT)]
        woT = [pp.tile([128, C], BF16, tag=f"woT{fb}", name=f"woT{fb}") for fb in range(NFB)]
        lout = [pp.tile([128, T], BF16, tag=f"lo{fb}", name=f"lo{fb}") for fb in range(NFB)]

        # =================================================================
        # Phase B/C: weight loads, x transposes, QKV projections
        # =================================================================
        with tc.tile_pool(name="pbc", bufs=2) as pbc, \
             tc.tile_pool(name="ps_tr", bufs=4, space="PSUM") as ps_tr, \
             tc.tile_pool(name="ps_acc", bufs=2, space="PSUM") as ps_acc:

            # bias broadcast [128, C] via rank-1 ones matmul (exact f32)
            for cc in range(2):
                bb = ps_acc.tile([128, 512], F32, tag="acc")
                nc.tensor.matmul(bb[:, :], ones_col[:, :],
                                 bo_row[:, cc * 512:(cc + 1) * 512],
                                 start=True, stop=True)
                nc.vector.tensor_copy(bo_bcast[:, cc * 512:(cc + 1) * 512], bb[:, :])

            # ---- weights: direct (host-pre-transposed) loads ------------
            def ctile_major(ext):
                return ext[:].rearrange("(ct p) f -> p ct f", p=128)

            # kTh[h]: head h's k at partitions (h%2)*64..+64, zeros in the
            # other half -> K=128 score matmuls with the full-qT rhs.
            for h in range(HPC):
                z0 = (1 - (h % 2)) * 64
                nc.gpsimd.memset(kTh[h][z0:z0 + 64, :], 0.0)

            # ---- xT: transpose x (bf16) into [C-part, tok] --------------
            # weight DMAs are interleaved after the first x tiles so the PE
            # gets transpose work immediately
            xT = [pbc.tile([128, T], BF16, tag=f"xT{ct}", name=f"xT{ct}", bufs=1) for ct in range(NCT)]
            wqTf = pbc.tile([128, NCT * FS], BF16, tag="wqTf", bufs=1)
            wkTf = pbc.tile([128, NCT * FS], BF16, tag="wkTf", bufs=1)
            wvT = pbc.tile([128, NCT * FS], BF16, tag="wvT", bufs=1)
            for tt in range(NT):
                xnat = pbc.tile([128, C], BF16, tag="xnat", bufs=3)
                nc.sync.dma_start(xnat[:, :], x_ext[tt * 128:(tt + 1) * 128, :])
                if tt == 2:
                    nc.sync.dma_start(wvT[:].rearrange("p (ct f) -> p ct f", f=FS),
                                      ctile_major(wvt_ext))
                elif tt == 4:
                    nc.sync.dma_start(wqTf[:].rearrange("p (ct f) -> p ct f", f=FS),
                                      ctile_major(wqt_ext))
                elif tt == 6:
                    nc.sync.dma_start(wkTf[:].rearrange("p (ct f) -> p ct f", f=FS),
                                      ctile_major(wkt_ext))
                elif tt == 8:
                    for fb in range(NFB):
                        nc.sync.dma_start(woT[fb][:, :],
                                          wot_ext[fb * 128:(fb + 1) * 128, :])
                for ct in range(NCT):
                    tr = ps_tr.tile([128, 128], BF16, tag="tr")
                    nc.tensor.transpose(tr[:, :], xnat[:, ct * 128:(ct + 1) * 128],
                                        identb[:, :])
                    nc.vector.tensor_copy(xT[ct][:, tt * 128:(tt + 1) * 128], tr[:, :])

            # ---- q/k projections ---------------------------------------
            for fb in range(NFB):
                for name, wf in (("wq", wqTf), ("wk", wkTf)):
                    for tch in range(NQC):
                        acc = ps_acc.tile([128, QW], F32, tag="acc")
                        for ct in range(NCT):
                            nc.tensor.matmul(
                                acc[:, :],
                                wf[:, ct * FS + fb * 128: ct * FS + fb * 128 + 128],
                                xT[ct][:, tch * QW:(tch + 1) * QW],
                                start=(ct == 0), stop=(ct == NCT - 1))
                        if name == "wq":
                            nc.vector.tensor_copy(
                                qT[fb][:, tch * QW:(tch + 1) * QW], acc[:, :])
                        else:
                            for hh in range(2):
                                nc.vector.tensor_copy(
                                    kTh[fb * 2 + hh][hh * 64:(hh + 1) * 64,
                                                     tch * QW:(tch + 1) * QW],
                                    acc[hh * 64:(hh + 1) * 64, :])

            # ---- v: natural [tok, feat] with ones column interleave -----
            for tt in range(NT):
                acc = ps_acc.tile([128, FS], F32, tag="acc")
                for ct in range(NCT):
                    nc.tensor.matmul(
                        acc[:, :],
                        xT[ct][:, tt * 128:(tt + 1) * 128],
                        wvT[:, ct * FS:(ct + 1) * FS],
                        start=(ct == 0), stop=(ct == NCT - 1))
                nc.gpsimd.memset(v_ext[tt][:, :], 1.0)
                dst = v_ext[tt][:].rearrange("p (h e) -> p h e", e=65)[:, :, 0:64]
                src = acc[:].rearrange("p (h e) -> p h e", e=64)
                nc.vector.tensor_copy(dst, src)

        # =================================================================
        # Phase D/E: attention + output projection + ReduceScatter
        # =================================================================
        with tc.tile_pool(name="pd", bufs=4) as pd, \
             tc.tile_pool(name="pdram", bufs=4, space="DRAM") as pdram, \
             tc.tile_pool(name="ps_sT", bufs=1, space="PSUM") as ps_sT, \
             tc.tile_pool(name="ps_oT", bufs=2, space="PSUM") as ps_oT, \
             tc.tile_pool(name="ps_misc", bufs=2, space="PSUM") as ps_misc:

            l_pad = pd.tile([128, QW], BF16, tag="l_pad", bufs=1, name="l_pad")
            nc.gpsimd.memset(l_pad[:, :], 0.0)
            # two alternating sT tiles (separate tensors -> independent WAR
            # chains; a single tile serializes every score matmul behind the
            # immediately preceding exp because reads are tracked per-tile)
            sTs = [ps_sT.tile([128, 1024], F32, tag=f"sT{i}", name=f"sT{i}", bufs=1)
                   for i in range(2)]
            gkp = [0]

            LAG = 3  # outT matmuls run LAG kp-iterations behind sT/exp
            pending_norm = []

            def attn(h, qc):
                fb, hh = divmod(h, 2)
                q_ap = qT[fb][:, qc * QW:(qc + 1) * QW]
                outT = ps_oT.tile([65, QW], F32, tag="outT")
                NKP = NT // 2
                pTs = {}

                def emit_outT(kp):
                    for j in range(2):
                        kt = kp * 2 + j
                        nc.tensor.matmul(
                            outT[:, :],
                            v_ext[kt][:, h * 65:(h + 1) * 65],
                            pTs[kp][:, j * 512:(j + 1) * 512],
                            start=(kp == 0 and j == 0),
                            stop=(kp == NKP - 1 and j == 1))

                for kp in range(NKP):
                    sT = sTs[gkp[0] % 2]
                    gkp[0] += 1
                    for j in range(2):
                        kt = kp * 2 + j
                        nc.tensor.matmul(
                            sT[:, j * 512:(j + 1) * 512],
                            kTh[h][:, kt * 128:(kt + 1) * 128],
                            q_ap, start=True, stop=True)
                    pT = pd.tile([128, 1024], BF16, tag="pT", bufs=5)
                    nc.scalar.activation(pT[:, :], sT[:, :], AF.Exp)
                    pTs[kp] = pT
                    if kp == 1 and pending_norm:
                        # previous head's normalization: inputs long since
                        # ready; emitting here keeps the PE stream stall-free
                        pending_norm.pop(0)()
                    if kp >= LAG:
                        emit_outT(kp - LAG)
                        del pTs[kp - LAG]
                for kp in range(NKP - LAG, NKP):
                    emit_outT(kp)

                def norm():
                    # broadcast denominators l across 64 partitions via the
                    # one-hot-row matmul, then a partition-parallel reciprocal
                    # (a [1,512] DVE op runs on one lane = ~3.4us; avoid it)
                    nc.scalar.copy(l_pad[0:1, :], outT[64:65, :])
                    rb_ps = ps_misc.tile([128, QW], F32, tag="misc", name="rb_ps")
                    nc.tensor.matmul(rb_ps[0:64, :], Emat[:, :], l_pad[:, :],
                                     start=True, stop=True)
                    rb = pd.tile([64, QW], F32, tag="rb_sb")
                    nc.vector.reciprocal_approx_fast(rb[:, :], rb_ps[0:64, :])
                    nc.vector.tensor_mul(
                        lout[fb][hh * 64:(hh + 1) * 64, qc * QW:(qc + 1) * QW],
                        outT[0:64, :], rb[:, :])
                pending_norm.append(norm)

            rs_mode = os.environ.get("KERNEL_RS_MODE", "two")
            if rs_mode == "one":
                rs_in_big = pdram.tile([T, C], BF16, tag="rs_in_big")
                rs_out_big = pdram.tile([T // 2, C], BF16, tag="rs_out_big")
            rs_chunks_done = []
            rs2_state = {}

            def proj_chunk(tq, rs_in, row_base):
                for half in range(2):
                    for t2 in range(2):
                        tok0 = half * 1024 + tq * 256 + t2 * 128
                        for cc in range(2):
                            pj = ps_misc.tile([128, 512], F32, tag="misc", name="pj")
                            for fb in range(NFB):
                                nc.tensor.matmul(
                                    pj[:, :],
                                    lout[fb][:, tok0:tok0 + 128],
                                    woT[fb][:, cc * 512:(cc + 1) * 512],
                                    start=(fb == 0), stop=(fb == NFB - 1))
                            ot = pd.tile([128, 512], BF16, tag="ot")
                            nc.vector.tensor_add(
                                ot[:, :], pj[:, :],
                                bo_bcast[:, cc * 512:(cc + 1) * 512])
                            r0 = row_base(half) + t2 * 128
                            nc.sync.dma_start(
                                rs_in[r0:r0 + 128, cc * 512:(cc + 1) * 512],
                                ot[:, :])

            def drain_rows(rs_out, src_row0, out_row0, nrows):
                for t2 in range(nrows // 128):
                    fo_bf = pd.tile([128, C], BF16, tag="fo_bf")
                    nc.sync.dma_start(
                        fo_bf[:, :],
                        rs_out[src_row0 + t2 * 128: src_row0 + (t2 + 1) * 128, :])
                    fo = pd.tile([128, C], F32, tag="fo")
                    nc.vector.tensor_copy(fo[:, :], fo_bf[:, :])
                    nc.sync.dma_start(
                        out_ext[out_row0 + t2 * 128: out_row0 + (t2 + 1) * 128, :],
                        fo[:, :])

            def proj_rs(tq):
                phases = os.environ.get("KERNEL_PHASES", "full")
                if rs_mode == "one":
                    proj_chunk(tq, rs_in_big,
                               lambda half: half * 1024 + tq * 256)
                    rs_chunks_done.append(tq)
                    if len(rs_chunks_done) == 4:
                        if phases == "nors":
                            nc.sync.dma_start(rs_out_big[:, :], rs_in_big[0:T // 2, :])
                        else:
                            nc.gpsimd.collective_compute(
                                "ReduceScatter", mybir.AluOpType.add,
                                replica_groups=PAIRS,
                                ins=[rs_in_big.opt()], outs=[rs_out_big.opt()])
                        drain_rows(rs_out_big, 0, 0, T // 2)
                elif rs_mode == "two":
                    # chunk i covers q-chunk pair (i, i+2): rows 0-511 =
                    # half0 tokens i*512..+512, rows 512-1023 = half1 same
                    i = tq // 2
                    if tq % 2 == 0:
                        self_rs = pdram.tile([1024, C], BF16, tag="rs_in2",
                                             name=f"rs_in2_{i}")
                        rs2_state[i] = self_rs
                    rs_in = rs2_state[i]
                    proj_chunk(tq, rs_in,
                               lambda half: half * 512 + (tq % 2) * 256)
                    if tq % 2 == 1:
                        rs_out = pdram.tile([512, C], BF16, tag="rs_out2",
                                            name=f"rs_out2_{i}")
                        if phases == "nors":
                            nc.sync.dma_start(rs_out[:, :], rs_in[0:512, :])
                        else:
                            nc.gpsimd.collective_compute(
                                "ReduceScatter", mybir.AluOpType.add,
                                replica_groups=PAIRS,
                                ins=[rs_in.opt()], outs=[rs_out.opt()])
                        drain_rows(rs_out, 0, i * 512, 512)
                else:
                    rs_in = pdram.tile([512, C], BF16, tag="rs_in", name="rs_in")
                    rs_out = pdram.tile([256, C], BF16, tag="rs_out", name="rs_out")
                    proj_chunk(tq, rs_in, lambda half: half * 256)
                    if phases == "nors":
                        nc.sync.dma_start(rs_out[:, :], rs_in[0:256, :])
                    else:
                        nc.gpsimd.collective_compute(
                            "ReduceScatter", mybir.AluOpType.add,
                            replica_groups=PAIRS,
                            ins=[rs_in.opt()], outs=[rs_out.opt()])
                    drain_rows(rs_out, 0, tq * 256, 256)

            phases = os.environ.get("KERNEL_PHASES", "full")
            if phases == "qkv":
                dbg = pd.tile([128, C], F32, tag="dbg")
                nc.vector.tensor_copy(dbg[:, :], qT[0][:, 0:1024])
                nc.sync.dma_start(out_ext[0:128, :], dbg[:, :])
            else:
                for pair_i, (qca, qcb) in enumerate(((0, 2), (1, 3))):
                    for qc in (qca, qcb):
                        for fb in range(NFB):
                            for hh in range(2):
                                attn(fb * 2 + hh, qc)
                    while pending_norm:
                        pending_norm.pop(0)()
                    if phases != "attn":
                        for tq in (pair_i * 2, pair_i * 2 + 1):
                            proj_rs(tq)
                if phases == "attn":
                    dbg = pd.tile([128, C], F32, tag="dbg")
                    for fb in range(NFB):
                        nc.vector.tensor_copy(dbg[:, :], lout[fb][:, 0:1024])
                        nc.sync.dma_start(out_ext[fb * 128:(fb + 1) * 128, :], dbg[:, :])


def _build_nc():
    nc = bacc.Bacc("TRN2", target_bir_lowering=False, debug=False,
                   num_devices=N_CORES)
    x_ext = nc.dram_tensor("x", [T, C], BF16, kind="ExternalInput")
    wqt_ext = nc.dram_tensor("wqt", [C, FS], BF16, kind="ExternalInput")
    wkt_ext = nc.dram_tensor("wkt", [C, FS], BF16, kind="ExternalInput")
    wvt_ext = nc.dram_tensor("wvt", [C, FS], BF16, kind="ExternalInput")
    wot_ext = nc.dram_tensor("wot", [FS, C], BF16, kind="ExternalInput")
    bo_ext = nc.dram_tensor("bo", [C], F32, kind="ExternalInput")
    out_ext = nc.dram_tensor("out", [T // 2, C], F32, kind="ExternalOutput")
    with tile.TileContext(nc) as tc:
        _emit(nc, tc, x_ext, wqt_ext, wkt_ext, wvt_ext, wot_ext, bo_ext, out_ext)
    nc.finalize()
    return nc


# ---------------------------------------------------------------------------
# NTFF profiling under axon (used when KERNEL_TRACE=1): the agent image's
# antenv lacks axon_hooks, so inject an equivalent module backed by the
# libaxon_pjrt.so profiling C ABI.
# ---------------------------------------------------------------------------
def _ensure_axon_hooks():
    try:
        from antenv.axon_hooks import get_axon_ntff_profile_hook  # noqa: F401
        return
    except ImportError:
        pass
    import ctypes
    import antenv

    so_path = "/opt/axon/libaxon_pjrt.so"
    lib = ctypes.CDLL(so_path)
    if not hasattr(lib, "axon_start_nrt_profile"):
        return
    lib.axon_start_nrt_profile.argtypes = [ctypes.POINTER(ctypes.c_int64),
                                           ctypes.c_size_t]
    lib.axon_start_nrt_profile.restype = ctypes.c_int64
    lib.axon_stop_nrt_profile.argtypes = [ctypes.c_char_p]
    lib.axon_stop_nrt_profile.restype = ctypes.c_int64

    @contextlib.contextmanager
    def _hook(output_dir, device_ids):
        import jax
        jax.devices()
        if device_ids:
            ids = (ctypes.c_int64 * len(device_ids))(*device_ids)
            rc = lib.axon_start_nrt_profile(ids, len(device_ids))
        else:
            rc = lib.axon_start_nrt_profile(None, 0)
        if rc != 0:
            raise RuntimeError(f"axon_start_nrt_profile rc={rc}")
        try:
            yield
        finally:
            n = lib.axon_stop_nrt_profile(str(output_dir).encode())
            print(f"ntff profile: {n} file(s) -> {output_dir}", file=sys.stderr)

    holder = [_hook]
    mod = types.ModuleType("antenv.axon_hooks")
    mod.get_axon_ntff_profile_hook = lambda: holder[0]
    mod.set_axon_ntff_profile_hook = lambda h: holder.__setitem__(0, h)
    sys.modules["antenv.axon_hooks"] = mod
    antenv.axon_hooks = mod
    # avoid S3 upload attempts during profile post-processing
    bass_utils.upload_artifacts = lambda tmpdir: f"(local:{tmpdir})"


_NC = None
LAST = {}


def kernel(hidden_states, wq, wk, wv, wo, bo):
    global _NC
    hidden_states = np.asarray(hidden_states, dtype=np.float32)
    wq = np.asarray(wq, dtype=np.float32)
    wk = np.asarray(wk, dtype=np.float32)
    wv = np.asarray(wv, dtype=np.float32)
    wo = np.asarray(wo, dtype=np.float32)
    bo = np.asarray(bo, dtype=np.float32)

    if _NC is None:
        _NC = _build_nc()

    bf = ml_dtypes.bfloat16
    scale = np.float32(D ** -0.5)
    in_maps = []
    for c in range(N_CORES):
        b, hg = divmod(c, 2)
        fr = hg * FS
        in_maps.append({
            "x": np.ascontiguousarray(hidden_states[b]).astype(bf),
            "wqt": np.ascontiguousarray((wq[fr:fr + FS] * scale).T).astype(bf),
            "wkt": np.ascontiguousarray(wk[fr:fr + FS].T).astype(bf),
            "wvt": np.ascontiguousarray(wv[fr:fr + FS].T).astype(bf),
            "wot": np.ascontiguousarray(wo[:, fr:fr + FS].T).astype(bf),
            "bo": bo * np.float32(0.5),
        })

    trace = os.environ.get("KERNEL_TRACE", "0") == "1"
    if trace:
        _ensure_axon_hooks()
    res = bass_utils.run_bass_kernel_spmd(
        _NC, in_maps, core_ids=list(range(N_CORES)), trace=trace)
    LAST["exec_time_ns"] = res.exec_time_ns
    LAST["res"] = res

    y = np.empty((B, T, C), dtype=np.float32)
    for c in range(N_CORES):
        b, hg = divmod(c, 2)
        y[b, hg * (T // 2):(hg + 1) * (T // 2), :] = res.results[c]["out"]
    return y


# revision 16
# speedup vs baseline: 1.7084x; 1.0129x over previous
"""Trainium2 Bass kernel for nn_Attention_84567906058480.

Multi-head attention (B=4, T=2048, C=1024, H=16, D=64) on 8 NeuronCores.

Sharding: core c = (batch b = c//2, head-group hg = c%2).  Each core computes
Q/K/V for its 8 heads over its batch (tensor-parallel split of wq/wk/wv rows),
runs attention, applies its column-slice of wo to get a partial output, and a
pairwise ReduceScatter (groups [2b, 2b+1]) sums the two head-group partials
while scattering token halves: the even core ends with tokens [0,1024) of its
batch, the odd core with tokens [1024,2048).  The host concatenates.

Implementation notes:
- Activations/weights run bf16 on the PE (f32 PSUM accumulate); rel-err ~5e-3.
- Weights are pre-transposed (and wq pre-scaled by 1/sqrt(D)) on the host and
  shipped bf16, so only x needs on-chip transposes.
- Scores are computed directly transposed (S.T = k.T-tiles @ qT) so no
  P-transpose is needed; the softmax denominator comes from a ones-column
  appended to V (M=65 stationary operand); exp needs no max-subtraction
  (|scores| < ~3 by construction).
- Every attention matmul contracts over K=128 (per-head K tensors are
  zero-padded into the other head's partition range) so the PE array never
  switches tiling modes; score and output matmul emission is software-
  pipelined (outputs lag scores by 2 iterations, normalization is deferred
  past the next head's start) to keep the PE stream stall-free.
- The output bias bo is halved on the host so the pairwise reduce adds it
  exactly once.
"""

import os
import sys
import types
import contextlib

import numpy as np

if "/opt/trn_rl_repo" not in sys.path:
    sys.path.insert(0, "/opt/trn_rl_repo")

import ml_dtypes
import concourse.bass as bass  # noqa: F401
import concourse.mybir as mybir
import concourse.tile as tile
from concourse import bacc
from concourse import bass_utils
from concourse.masks import make_identity

F32 = mybir.dt.float32
BF16 = mybir.dt.bfloat16
AF = mybir.ActivationFunctionType

B, T, C = 4, 2048, 1024
H, D = 16, 64
HPC = 8            # heads per core
FS = HPC * D       # per-core feature shard = 512
N_CORES = 8
PAIRS = [[0, 1], [2, 3], [4, 5], [6, 7]]

NT = T // 128      # 16 token tiles
NCT = C // 128     # 8 contraction tiles
NFB = FS // 128    # 4 feature blocks per core
QW = 512           # q chunk width
NQC = T // QW      # 4 q chunks


def _emit(nc, tc, x_ext, wqt_ext, wkt_ext, wvt_ext, wot_ext, bo_ext, out_ext):
    with tc.tile_pool(name="const", bufs=1) as constp, \
         tc.tile_pool(name="persist", bufs=1) as pp:

        # ---- constants -------------------------------------------------
        identb = constp.tile([128, 128], BF16, tag="identb")
        make_identity(nc, identb[:, :])
        ones_col = constp.tile([1, 128], F32, tag="ones")
        nc.gpsimd.memset(ones_col[:, :], 1.0)
        Emat = constp.tile([128, 64], BF16, tag="Emat")
        nc.gpsimd.memset(Emat[:, :], 0.0)
        nc.gpsimd.memset(Emat[0:1, :], 1.0)
        bo_row = constp.tile([1, C], F32, tag="bo_row")
        nc.sync.dma_start(bo_row[:, :], bo_ext[:].unsqueeze(0))
        bo_bcast = constp.tile([128, C], F32, tag="bo_bcast")

        # ---- persistent activation storage (bf16) ----------------------
        qT = [pp.tile([128, T], BF16, tag=f"qT{fb}", name=f"qT{fb}") for fb in range(NFB)]
        kTh = [pp.tile([128, T], BF16, tag=f"kTh{h}", name=f"kTh{h}") for h in range(HPC)]
        v_ext = [pp.tile([128, HPC * 65], BF16, tag=f"vx{tt}", name=f"vx{tt}") for tt in range(NT)]
        woT = [pp.tile([128, C], BF16, tag=f"woT{fb}", name=f"woT{fb}") for fb in range(NFB)]
        lout = [pp.tile([128, T], BF16, tag=f"lo{fb}", name=f"lo{fb}") for fb in range(NFB)]

        # =================================================================
        # Phase B/C: weight loads, x transposes, QKV projections
        # =================================================================
        with tc.tile_pool(name="pbc", bufs=2) as pbc, \
             tc.tile_pool(name="ps_tr", bufs=4, space="PSUM") as ps_tr, \
             tc.tile_pool(name="ps_acc", bufs=2, space="PSUM") as ps_acc:

            # bias broadcast [128, C] via rank-1 ones matmul (exact f32)
            for cc in range(2):
                bb = ps_acc.tile([128, 512], F32, tag="acc")
                nc.tensor.matmul(bb[:, :], ones_col[:, :],
                                 bo_row[:, cc * 512:(cc + 1) * 512],
                                 start=True, stop=True)
                nc.vector.tensor_copy(bo_bcast[:, cc * 512:(cc + 1) * 512], bb[:, :])

            # ---- weights: direct (host-pre-transposed) loads ------------
            def ctile_major(ext):
                return ext[:].rearrange("(ct p) f -> p ct f", p=128)

            # kTh[h]: head h's k at partitions (h%2)*64..+64, zeros in the
            # other half -> K=128 score matmuls with the full-qT rhs.
            for h in range(HPC):
                z0 = (1 - (h % 2)) * 64
                nc.gpsimd.memset(kTh[h][z0:z0 + 64, :], 0.0)

            # ---- xT: transpose x (bf16) into [C-part, tok] --------------
            # weight DMAs are interleaved after the first x tiles so the PE
            # gets transpose work immediately
            xT = [pbc.tile([128, T], BF16, tag=f"xT{ct}", name=f"xT{ct}", bufs=1) for ct in range(NCT)]
            wqTf = pbc.tile([128, NCT * FS], BF16, tag="wqTf", bufs=1)
            wkTf = pbc.tile([128, NCT * FS], BF16, tag="wkTf", bufs=1)
            wvT = pbc.tile([128, NCT * FS], BF16, tag="wvT", bufs=1)
            for tp in range(NT // 2):
                xnats = []
                for j in range(2):
                    tt = tp * 2 + j
                    xnat = pbc.tile([128, C], BF16, tag="xnat", bufs=4, name="xnat")
                    nc.sync.dma_start(xnat[:, :], x_ext[tt * 128:(tt + 1) * 128, :])
                    xnats.append(xnat)
                if tp == 1:
                    nc.sync.dma_start(wvT[:].rearrange("p (ct f) -> p ct f", f=FS),
                                      ctile_major(wvt_ext))
                elif tp == 2:
                    nc.sync.dma_start(wqTf[:].rearrange("p (ct f) -> p ct f", f=FS),
                                      ctile_major(wqt_ext))
                elif tp == 3:
                    nc.sync.dma_start(wkTf[:].rearrange("p (ct f) -> p ct f", f=FS),
                                      ctile_major(wkt_ext))
                elif tp == 4:
                    for fb in range(NFB):
                        nc.sync.dma_start(woT[fb][:, :],
                                          wot_ext[fb * 128:(fb + 1) * 128, :])
                for ct in range(NCT):
                    tr = ps_tr.tile([128, 256], BF16, tag="tr")
                    for j in range(2):
                        nc.tensor.transpose(tr[:, j * 128:(j + 1) * 128],
                                            xnats[j][:, ct * 128:(ct + 1) * 128],
                                            identb[:, :])
                    nc.scalar.copy(xT[ct][:, tp * 256:(tp + 1) * 256], tr[:, :])

            # ---- q/k projections ---------------------------------------
            for fb in range(NFB):
                for name, wf in (("wq", wqTf), ("wk", wkTf)):
                    for tch in range(NQC):
                        acc = ps_acc.tile([128, QW], F32, tag="acc")
                        for ct in range(NCT):
                            nc.tensor.matmul(
                                acc[:, :],
                                wf[:, ct * FS + fb * 128: ct * FS + fb * 128 + 128],
                                xT[ct][:, tch * QW:(tch + 1) * QW],
                                start=(ct == 0), stop=(ct == NCT - 1))
                        if name == "wq":
                            nc.scalar.copy(
                                qT[fb][:, tch * QW:(tch + 1) * QW], acc[:, :])
                        else:
                            for hh in range(2):
                                nc.vector.tensor_copy(
                                    kTh[fb * 2 + hh][hh * 64:(hh + 1) * 64,
                                                     tch * QW:(tch + 1) * QW],
                                    acc[hh * 64:(hh + 1) * 64, :])

            # ---- v: natural [tok, feat] with ones column interleave -----
            for tt in range(NT):
                acc = ps_acc.tile([128, FS], F32, tag="acc")
                for ct in range(NCT):
                    nc.tensor.matmul(
                        acc[:, :],
                        xT[ct][:, tt * 128:(tt + 1) * 128],
                        wvT[:, ct * FS:(ct + 1) * FS],
                        start=(ct == 0), stop=(ct == NCT - 1))
                nc.gpsimd.memset(v_ext[tt][:, :], 1.0)
                dst = v_ext[tt][:].rearrange("p (h e) -> p h e", e=65)[:, :, 0:64]
                src = acc[:].rearrange("p (h e) -> p h e", e=64)
                nc.vector.tensor_copy(dst, src)

        # =================================================================
        # Phase D/E: attention + output projection + ReduceScatter
        # =================================================================
        with tc.tile_pool(name="pd", bufs=4) as pd, \
             tc.tile_pool(name="pdram", bufs=4, space="DRAM") as pdram, \
             tc.tile_pool(name="ps_sT", bufs=1, space="PSUM") as ps_sT, \
             tc.tile_pool(name="ps_oT", bufs=2, space="PSUM") as ps_oT, \
             tc.tile_pool(name="ps_misc", bufs=2, space="PSUM") as ps_misc:

            l_pad = pd.tile([128, QW], BF16, tag="l_pad", bufs=1, name="l_pad")
            nc.gpsimd.memset(l_pad[:, :], 0.0)
            # two alternating sT tiles (separate tensors -> independent WAR
            # chains; a single tile serializes every score matmul behind the
            # immediately preceding exp because reads are tracked per-tile)
            sTs = [ps_sT.tile([128, 1024], F32, tag=f"sT{i}", name=f"sT{i}", bufs=1)
                   for i in range(2)]
            gkp = [0]

            LAG = 3  # outT matmuls run LAG kp-iterations behind sT/exp
            pending_norm = []

            def attn(h, qc):
                fb, hh = divmod(h, 2)
                q_ap = qT[fb][:, qc * QW:(qc + 1) * QW]
                outT = ps_oT.tile([65, QW], F32, tag="outT")
                NKP = NT // 2
                pTs = {}

                def emit_outT(kp):
                    for j in range(2):
                        kt = kp * 2 + j
                        nc.tensor.matmul(
                            outT[:, :],
                            v_ext[kt][:, h * 65:(h + 1) * 65],
                            pTs[kp][:, j * 512:(j + 1) * 512],
                            start=(kp == 0 and j == 0),
                            stop=(kp == NKP - 1 and j == 1))

                for kp in range(NKP):
                    sT = sTs[gkp[0] % 2]
                    gkp[0] += 1
                    for j in range(2):
                        kt = kp * 2 + j
                        nc.tensor.matmul(
                            sT[:, j * 512:(j + 1) * 512],
                            kTh[h][:, kt * 128:(kt + 1) * 128],
                            q_ap, start=True, stop=True)
                    pT = pd.tile([128, 1024], BF16, tag="pT", bufs=5)
                    nc.scalar.activation(pT[:, :], sT[:, :], AF.Exp)
                    pTs[kp] = pT
                    if kp == 1 and pending_norm:
                        # previous head's normalization: inputs long since
                        # ready; emitting here keeps the PE stream stall-free
                        pending_norm.pop(0)()
                    if kp >= LAG:
                        emit_outT(kp - LAG)
                        del pTs[kp - LAG]
                for kp in range(NKP - LAG, NKP):
                    emit_outT(kp)

                def norm():
                    # broadcast denominators l across 64 partitions via the
                    # one-hot-row matmul, then a partition-parallel reciprocal
                    # (a [1,512] DVE op runs on one lane = ~3.4us; avoid it)
                    nc.scalar.copy(l_pad[0:1, :], outT[64:65, :])
                    rb_ps = ps_misc.tile([128, QW], F32, tag="misc", name="rb_ps")
                    nc.tensor.matmul(rb_ps[0:64, :], Emat[:, :], l_pad[:, :],
                                     start=True, stop=True)
                    rb = pd.tile([64, QW], F32, tag="rb_sb")
                    nc.vector.reciprocal_approx_fast(rb[:, :], rb_ps[0:64, :])
                    nc.vector.tensor_mul(
                        lout[fb][hh * 64:(hh + 1) * 64, qc * QW:(qc + 1) * QW],
                        outT[0:64, :], rb[:, :])
                pending_norm.append(norm)

            rs_mode = os.environ.get("KERNEL_RS_MODE", "two")
            if rs_mode == "one":
                rs_in_big = pdram.tile([T, C], BF16, tag="rs_in_big")
                rs_out_big = pdram.tile([T // 2, C], BF16, tag="rs_out_big")
            rs_chunks_done = []
            rs2_state = {}

            def proj_chunk(tq, rs_in, row_base):
                for half in range(2):
                    for t2 in range(2):
                        tok0 = half * 1024 + tq * 256 + t2 * 128
                        for cc in range(2):
                            pj = ps_misc.tile([128, 512], F32, tag="misc", name="pj")
                            for fb in range(NFB):
                                nc.tensor.matmul(
                                    pj[:, :],
                                    lout[fb][:, tok0:tok0 + 128],
                                    woT[fb][:, cc * 512:(cc + 1) * 512],
                                    start=(fb == 0), stop=(fb == NFB - 1))
                            ot = pd.tile([128, 512], BF16, tag="ot")
                            nc.vector.tensor_add(
                                ot[:, :], pj[:, :],
                                bo_bcast[:, cc * 512:(cc + 1) * 512])
                            r0 = row_base(half) + t2 * 128
                            nc.sync.dma_start(
                                rs_in[r0:r0 + 128, cc * 512:(cc + 1) * 512],
                                ot[:, :])

            def drain_rows(rs_out, src_row0, out_row0, nrows):
                for t2 in range(nrows // 128):
                    fo_bf = pd.tile([128, C], BF16, tag="fo_bf")
                    nc.sync.dma_start(
                        fo_bf[:, :],
                        rs_out[src_row0 + t2 * 128: src_row0 + (t2 + 1) * 128, :])
                    fo = pd.tile([128, C], F32, tag="fo")
                    nc.gpsimd.tensor_copy(fo[:, :], fo_bf[:, :])
                    nc.sync.dma_start(
                        out_ext[out_row0 + t2 * 128: out_row0 + (t2 + 1) * 128, :],
                        fo[:, :])

            def proj_rs(tq):
                phases = os.environ.get("KERNEL_PHASES", "full")
                if rs_mode == "one":
                    proj_chunk(tq, rs_in_big,
                               lambda half: half * 1024 + tq * 256)
                    rs_chunks_done.append(tq)
                    if len(rs_chunks_done) == 4:
                        if phases == "nors":
                            nc.sync.dma_start(rs_out_big[:, :], rs_in_big[0:T // 2, :])
                        else:
                            nc.gpsimd.collective_compute(
                                "ReduceScatter", mybir.AluOpType.add,
                                replica_groups=PAIRS,
                                ins=[rs_in_big.opt()], outs=[rs_out_big.opt()])
                        drain_rows(rs_out_big, 0, 0, T // 2)
                elif rs_mode == "two":
                    # chunk i covers q-chunk pair (i, i+2): rows 0-511 =
                    # half0 tokens i*512..+512, rows 512-1023 = half1 same
                    i = tq // 2
                    if tq % 2 == 0:
                        self_rs = pdram.tile([1024, C], BF16, tag="rs_in2",
                                             name=f"rs_in2_{i}")
                        rs2_state[i] = self_rs
                    rs_in = rs2_state[i]
                    proj_chunk(tq, rs_in,
                               lambda half: half * 512 + (tq % 2) * 256)
                    if tq % 2 == 1:
                        rs_out = pdram.tile([512, C], BF16, tag="rs_out2",
                                            name=f"rs_out2_{i}")
                        if phases == "nors":
                            nc.sync.dma_start(rs_out[:, :], rs_in[0:512, :])
                        else:
                            nc.gpsimd.collective_compute(
                                "ReduceScatter", mybir.AluOpType.add,
                                replica_groups=PAIRS,
                                ins=[rs_in.opt()], outs=[rs_out.opt()])
                        drain_rows(rs_out, 0, i * 512, 512)
                else:
                    rs_in = pdram.tile([512, C], BF16, tag="rs_in", name="rs_in")
                    rs_out = pdram.tile([256, C], BF16, tag="rs_out", name="rs_out")
                    proj_chunk(tq, rs_in, lambda half: half * 256)
                    if phases == "nors":
                        nc.sync.dma_start(rs_out[:, :], rs_in[0:256, :])
                    else:
                        nc.gpsimd.collective_compute(
                            "ReduceScatter", mybir.AluOpType.add,
                            replica_groups=PAIRS,
                            ins=[rs_in.opt()], outs=[rs_out.opt()])
                    drain_rows(rs_out, 0, tq * 256, 256)

            phases = os.environ.get("KERNEL_PHASES", "full")
            if phases == "qkv":
                dbg = pd.tile([128, C], F32, tag="dbg")
                nc.vector.tensor_copy(dbg[:, :], qT[0][:, 0:1024])
                nc.sync.dma_start(out_ext[0:128, :], dbg[:, :])
            else:
                for pair_i, (qca, qcb) in enumerate(((0, 2), (1, 3))):
                    for qc in (qca, qcb):
                        for fb in range(NFB):
                            for hh in range(2):
                                attn(fb * 2 + hh, qc)
                    while pending_norm:
                        pending_norm.pop(0)()
                    if phases != "attn":
                        for tq in (pair_i * 2, pair_i * 2 + 1):
                            proj_rs(tq)
                if phases == "attn":
                    dbg = pd.tile([128, C], F32, tag="dbg")
                    for fb in range(NFB):
                        nc.vector.tensor_copy(dbg[:, :], lout[fb][:, 0:1024])
                        nc.sync.dma_start(out_ext[fb * 128:(fb + 1) * 128, :], dbg[:, :])


def _build_nc():
    nc = bacc.Bacc("TRN2", target_bir_lowering=False, debug=False,
                   num_devices=N_CORES)
    x_ext = nc.dram_tensor("x", [T, C], BF16, kind="ExternalInput")
    wqt_ext = nc.dram_tensor("wqt", [C, FS], BF16, kind="ExternalInput")
    wkt_ext = nc.dram_tensor("wkt", [C, FS], BF16, kind="ExternalInput")
    wvt_ext = nc.dram_tensor("wvt", [C, FS], BF16, kind="ExternalInput")
    wot_ext = nc.dram_tensor("wot", [FS, C], BF16, kind="ExternalInput")
    bo_ext = nc.dram_tensor("bo", [C], F32, kind="ExternalInput")
    out_ext = nc.dram_tensor("out", [T // 2, C], F32, kind="ExternalOutput")
    with tile.TileContext(nc) as tc:
        _emit(nc, tc, x_ext, wqt_ext, wkt_ext, wvt_ext, wot_ext, bo_ext, out_ext)
    nc.finalize()
    return nc


# ---------------------------------------------------------------------------
# NTFF profiling under axon (used when KERNEL_TRACE=1): the agent image's
# antenv lacks axon_hooks, so inject an equivalent module backed by the
# libaxon_pjrt.so profiling C ABI.
# ---------------------------------------------------------------------------
def _ensure_axon_hooks():
    try:
        from antenv.axon_hooks import get_axon_ntff_profile_hook  # noqa: F401
        return
    except ImportError:
        pass
    import ctypes
    import antenv

    so_path = "/opt/axon/libaxon_pjrt.so"
    lib = ctypes.CDLL(so_path)
    if not hasattr(lib, "axon_start_nrt_profile"):
        return
    lib.axon_start_nrt_profile.argtypes = [ctypes.POINTER(ctypes.c_int64),
                                           ctypes.c_size_t]
    lib.axon_start_nrt_profile.restype = ctypes.c_int64
    lib.axon_stop_nrt_profile.argtypes = [ctypes.c_char_p]
    lib.axon_stop_nrt_profile.restype = ctypes.c_int64

    @contextlib.contextmanager
    def _hook(output_dir, device_ids):
        import jax
        jax.devices()
        if device_ids:
            ids = (ctypes.c_int64 * len(device_ids))(*device_ids)
            rc = lib.axon_start_nrt_profile(ids, len(device_ids))
        else:
            rc = lib.axon_start_nrt_profile(None, 0)
        if rc != 0:
            raise RuntimeError(f"axon_start_nrt_profile rc={rc}")
        try:
            yield
        finally:
            n = lib.axon_stop_nrt_profile(str(output_dir).encode())
            print(f"ntff profile: {n} file(s) -> {output_dir}", file=sys.stderr)

    holder = [_hook]
    mod = types.ModuleType("antenv.axon_hooks")
    mod.get_axon_ntff_profile_hook = lambda: holder[0]
    mod.set_axon_ntff_profile_hook = lambda h: holder.__setitem__(0, h)
    sys.modules["antenv.axon_hooks"] = mod
    antenv.axon_hooks = mod
    # avoid S3 upload attempts during profile post-processing
    bass_utils.upload_artifacts = lambda tmpdir: f"(local:{tmpdir})"


_NC = None
LAST = {}


def kernel(hidden_states, wq, wk, wv, wo, bo):
    global _NC
    hidden_states = np.asarray(hidden_states, dtype=np.float32)
    wq = np.asarray(wq, dtype=np.float32)
    wk = np.asarray(wk, dtype=np.float32)
    wv = np.asarray(wv, dtype=np.float32)
    wo = np.asarray(wo, dtype=np.float32)
    bo = np.asarray(bo, dtype=np.float32)

    if _NC is None:
        _NC = _build_nc()

    bf = ml_dtypes.bfloat16
    scale = np.float32(D ** -0.5)
    in_maps = []
    for c in range(N_CORES):
        b, hg = divmod(c, 2)
        fr = hg * FS
        in_maps.append({
            "x": np.ascontiguousarray(hidden_states[b]).astype(bf),
            "wqt": np.ascontiguousarray((wq[fr:fr + FS] * scale).T).astype(bf),
            "wkt": np.ascontiguousarray(wk[fr:fr + FS].T).astype(bf),
            "wvt": np.ascontiguousarray(wv[fr:fr + FS].T).astype(bf),
            "wot": np.ascontiguousarray(wo[:, fr:fr + FS].T).astype(bf),
            "bo": bo * np.float32(0.5),
        })

    trace = os.environ.get("KERNEL_TRACE", "0") == "1"
    if trace:
        _ensure_axon_hooks()
    res = bass_utils.run_bass_kernel_spmd(
        _NC, in_maps, core_ids=list(range(N_CORES)), trace=trace)
    LAST["exec_time_ns"] = res.exec_time_ns
    LAST["res"] = res

    y = np.empty((B, T, C), dtype=np.float32)
    for c in range(N_CORES):
        b, hg = divmod(c, 2)
        y[b, hg * (T // 2):(hg + 1) * (T // 2), :] = res.results[c]["out"]
    return y
